# revision 21
# baseline (speedup 1.0000x reference)
"""ConceptRWKV Trainium2 kernel (8 NeuronCores, SPMD).

Sharding:
  - Trunk: sequence-parallel. Core c handles batch c//4, token chunk
    (c%4)*512. Residual h kept feature-major [D(4x128 part-tiles), 512 tok]
    in SBUF float32r. Per-layer cross-core exchange: decay-cumsum carry
    (subgroup AllGather of local totals + mask matmul) and GroupNorm
    global stats (subgroup AllReduce).
  - Head: V-sharded. concepts [64, 4096] AllGathered across all 8 cores;
    each core computes logits for its own 4000-column V slice.
Host reassembles logits by concatenating V slices.
"""

import numpy as np

import concourse.bass as bass
import concourse.mybir as mybir
import concourse.tile as tile
from concourse import bacc
from concourse.bass_utils import run_bass_kernel_spmd
from concourse.masks import make_identity

B, T, V, D, FF, L, CD = 2, 2048, 32000, 512, 2048, 6, 64
EPS = 1e-5
NC = 8
GP = 4          # cores per batch group
TOK = 512       # tokens per core
NT = TOK // 128  # token tiles per core
ND = D // 128    # d tiles
NF = FF // 128   # ff tiles
VS = V // NC     # vocab slice per core
NVC = 8
VCW = VS // NVC  # 500, one psum bank
NTT = (B * T) // 128  # 32 token tiles over the full stream

f32 = mybir.dt.float32
f32r = mybir.dt.float32r
i32 = mybir.dt.int32
AF = mybir.ActivationFunctionType
ALU = mybir.AluOpType

GROUPS4 = [[0, 1, 2, 3], [4, 5, 6, 7]]
GROUPS8 = [list(range(NC))]


def _build():
    import os
    ABL = set(os.environ.get("RWKV_ABLATE", "").split(","))
    nc = bacc.Bacc("TRN2", target_bir_lowering=False, debug=False,
                   num_devices=NC)

    # ---- parameters ----
    embed = nc.declare_dram_parameter("embed", [V, D], f32, isOutput=False)
    idxs = nc.declare_dram_parameter("idxs", [TOK, 1], i32, isOutput=False)
    tglob = nc.declare_dram_parameter("tglob", [128, NT], f32, isOutput=False)
    gmask = nc.declare_dram_parameter("gmask", [GP, 1], f32r, isOutput=False)
    tri = nc.declare_dram_parameter("tri", [128, 128], f32r, isOutput=False)
    onescol = nc.declare_dram_parameter("onescol", [128, 1], f32r, isOutput=False)
    onerow = nc.declare_dram_parameter("onerow", [1, 128], f32r, isOutput=False)
    onessq = nc.declare_dram_parameter("onessq", [128, 128], f32r, isOutput=False)
    onesrow4k = nc.declare_dram_parameter("onesrow4k", [1, B * T], f32r,
                                          isOutput=False)
    lnig = nc.declare_dram_parameter("lnig", [128, D], f32, isOutput=False)
    lnib = nc.declare_dram_parameter("lnib", [128, D], f32, isOutput=False)
    lnog = nc.declare_dram_parameter("lnog", [D, 1], f32, isOutput=False)
    lnob = nc.declare_dram_parameter("lnob", [D, 1], f32, isOutput=False)
    encW = nc.declare_dram_parameter("encW", [D, CD], f32r, isOutput=False)
    encB = nc.declare_dram_parameter("encB", [CD, 1], f32, isOutput=False)
    decW = nc.declare_dram_parameter("decW", [CD, VS], f32r, isOutput=False)
    decB = nc.declare_dram_parameter("decB", [1, VS], f32r, isOutput=False)

    Wr, Wk, Wv, Wo, W1, W2, Wf = [], [], [], [], [], [], []
    ln1g, ln1b, ln2g, ln2b, gng, gnb, decs = [], [], [], [], [], [], []
    for l in range(L):
        Wr.append(nc.declare_dram_parameter(f"Wr{l}", [D, D], f32r, isOutput=False))
        Wk.append(nc.declare_dram_parameter(f"Wk{l}", [D, D], f32r, isOutput=False))
        Wv.append(nc.declare_dram_parameter(f"Wv{l}", [D, D], f32r, isOutput=False))
        Wo.append(nc.declare_dram_parameter(f"Wo{l}", [D, D], f32r, isOutput=False))
        W1.append(nc.declare_dram_parameter(f"W1{l}", [D, FF], f32r, isOutput=False))
        W2.append(nc.declare_dram_parameter(f"W2{l}", [D, FF], f32r, isOutput=False))
        Wf.append(nc.declare_dram_parameter(f"Wf{l}", [FF, D], f32r, isOutput=False))
        ln1g.append(nc.declare_dram_parameter(f"ln1g{l}", [D, 1], f32, isOutput=False))
        ln1b.append(nc.declare_dram_parameter(f"ln1b{l}", [D, 1], f32, isOutput=False))
        ln2g.append(nc.declare_dram_parameter(f"ln2g{l}", [D, 1], f32, isOutput=False))
        ln2b.append(nc.declare_dram_parameter(f"ln2b{l}", [D, 1], f32, isOutput=False))
        gng.append(nc.declare_dram_parameter(f"gng{l}", [128, D], f32, isOutput=False))
        gnb.append(nc.declare_dram_parameter(f"gnb{l}", [128, D], f32, isOutput=False))
        decs.append(nc.declare_dram_parameter(f"dec{l}", [128, D], f32, isOutput=False))

    logits = nc.declare_dram_parameter("logits", [B * T, VS], f32, isOutput=True)

    with tile.TileContext(nc) as tc:
        with (
            tc.tile_pool(name="const", bufs=1) as cp,
            tc.tile_pool(name="persist", bufs=1) as hp,
            tc.tile_pool(name="big", bufs=1) as bg,
            tc.tile_pool(name="tmp", bufs=2) as tp,
            tc.tile_pool(name="rows", bufs=1) as rp,
            tc.tile_pool(name="ppmm", bufs=2, space="PSUM") as ppm,
            tc.tile_pool(name="ppacc", bufs=4, space="PSUM") as ppa,
            tc.tile_pool(name="ppst", bufs=2, space="PSUM") as pps,
            tc.tile_pool(name="dram", bufs=1, space="DRAM") as dr,
        ):
            # ---- constants ----
            ident = cp.tile([128, 128], f32)
            make_identity(nc, ident[:])
            tri_sb = cp.tile([128, 128], f32r)
            nc.sync.dma_start(tri_sb[:], tri[:])
            ones_sq = cp.tile([128, 128], f32r)
            nc.sync.dma_start(ones_sq[:], onessq[:])
            ones_col = cp.tile([128, 1], f32r)
            nc.sync.dma_start(ones_col[:], onescol[:])
            one_row = cp.tile([1, 128], f32r)
            nc.sync.dma_start(one_row[:], onerow[:])
            gmask_sb = cp.tile([GP, 1], f32r)
            nc.sync.dma_start(gmask_sb[:], gmask[:])
            tglob_sb = cp.tile([128, NT], f32)
            nc.sync.dma_start(tglob_sb[:], tglob[:])
            epsc = cp.tile([128, 1], f32)
            nc.vector.memset(epsc[:], EPS)

            # persistent residual stream, feature-major f32r
            h = [hp.tile([128, TOK], f32r, tag=f"h{d}", name=f"h{d}") for d in range(ND)]

            # ---------- helpers ----------
            def ln_fm(hin, g_dram, b_dram, out_tiles):
                """LayerNorm over D (partitions) of feature-major hin -> f32r."""
                # per-token sum / sumsq via ones-matmuls
                ps_s = pps.tile([1, TOK], f32, tag="st")
                ps_q = pps.tile([1, TOK], f32, tag="st")
                for d in range(ND):
                    nc.tensor.matmul(ps_s[:], lhsT=ones_col[:], rhs=hin[d][:],
                                     start=(d == 0), stop=(d == ND - 1))
                for d in range(ND):
                    hsq = tp.tile([128, TOK], f32r, tag="tr1")
                    nc.gpsimd.tensor_tensor(hsq[:], hin[d][:], hin[d][:],
                                             op=ALU.mult)
                    nc.tensor.matmul(ps_q[:], lhsT=ones_col[:], rhs=hsq[:],
                                     start=(d == 0), stop=(d == ND - 1))
                mean_r = rp.tile([1, TOK], f32, tag="mean")
                nc.vector.tensor_scalar_mul(mean_r[:], ps_s[:], 1.0 / D)
                msq_r = rp.tile([1, TOK], f32, tag="msq")
                nc.vector.tensor_scalar_mul(msq_r[:], ps_q[:], 1.0 / D)
                m2_r = rp.tile([1, TOK], f32, tag="m2")
                nc.vector.tensor_tensor(m2_r[:], mean_r[:], mean_r[:], op=ALU.mult)
                var_r = rp.tile([1, TOK], f32, tag="var")
                nc.vector.tensor_tensor(var_r[:], msq_r[:], m2_r[:],
                                        op=ALU.subtract)
                nc.scalar.activation(var_r[:], var_r[:], AF.Sqrt, bias=epsc[:1, :])
                rstd_r = rp.tile([1, TOK], f32, tag="rstd")
                nc.vector.reciprocal(rstd_r[:], var_r[:])
                # A = rstd, Bc = -mean*rstd   (f32r rows for K=1 bcast matmul)
                a_r = rp.tile([1, TOK], f32r, tag="arow")
                nc.vector.tensor_copy(a_r[:], rstd_r[:])
                b_t = rp.tile([1, TOK], f32, tag="btmp")
                nc.vector.tensor_tensor(b_t[:], mean_r[:], rstd_r[:], op=ALU.mult)
                b_r = rp.tile([1, TOK], f32r, tag="brow")
                nc.vector.tensor_scalar_mul(b_r[:], b_t[:], -1.0)
                ps_a = pps.tile([128, TOK], f32, tag="st")
                nc.tensor.matmul(ps_a[:], lhsT=one_row[:], rhs=a_r[:],
                                 start=True, stop=True)
                ps_b = pps.tile([128, TOK], f32, tag="st")
                nc.tensor.matmul(ps_b[:], lhsT=one_row[:], rhs=b_r[:],
                                 start=True, stop=True)
                for d in range(ND):
                    gc = rp.tile([128, 1], f32, tag="gcol")
                    nc.sync.dma_start(gc[:], g_dram[d * 128:(d + 1) * 128, :])
                    bc = rp.tile([128, 1], f32, tag="bcol")
                    nc.sync.dma_start(bc[:], b_dram[d * 128:(d + 1) * 128, :])
                    t1 = tp.tile([128, TOK], f32, tag="ta")
                    nc.vector.tensor_tensor(t1[:], hin[d][:], ps_a[:], op=ALU.mult)
                    t2 = tp.tile([128, TOK], f32, tag="tb")
                    nc.vector.tensor_tensor(t2[:], t1[:], ps_b[:], op=ALU.add)
                    nc.scalar.activation(out_tiles[d][:], t2[:], AF.Identity,
                                         bias=bc[:], scale=gc[:])

            # ---------- embedding + input LN ----------
            x = [bg.tile([128, TOK], f32r, tag=f"x{d}", name=f"x{d}") for d in range(ND)]
            lnig_sb = bg.tile([128, D], f32, tag="gfull")
            nc.sync.dma_start(lnig_sb[:], lnig[:])
            lnib_sb = bg.tile([128, D], f32, tag="bfull")
            nc.sync.dma_start(lnib_sb[:], lnib[:])
            for tt in range(NT):
                idx_t = tp.tile([128, 1], i32, tag="idxt")
                nc.sync.dma_start(idx_t[:], idxs[tt * 128:(tt + 1) * 128, :])
                emb = tp.tile([128, D], f32, tag="ta")
                nc.gpsimd.indirect_dma_start(
                    out=emb[:], out_offset=None, in_=embed[:],
                    in_offset=bass.IndirectOffsetOnAxis(ap=idx_t[:, :1], axis=0))
                s1 = rp.tile([128, 1], f32, tag="es1")
                nc.vector.tensor_reduce(s1[:], emb[:], axis=mybir.AxisListType.X,
                                        op=ALU.add)
                nc.vector.tensor_scalar_mul(s1[:], s1[:], 1.0 / D)
                sq = tp.tile([128, D], f32, tag="tb")
                nc.vector.tensor_tensor(sq[:], emb[:], emb[:], op=ALU.mult)
                s2 = rp.tile([128, 1], f32, tag="es2")
                nc.vector.tensor_reduce(s2[:], sq[:], axis=mybir.AxisListType.X,
                                        op=ALU.add)
                nc.vector.tensor_scalar_mul(s2[:], s2[:], 1.0 / D)
                m2 = rp.tile([128, 1], f32, tag="em2")
                nc.vector.tensor_tensor(m2[:], s1[:], s1[:], op=ALU.mult)
                vv = rp.tile([128, 1], f32, tag="evv")
                nc.vector.tensor_tensor(vv[:], s2[:], m2[:], op=ALU.subtract)
                nc.scalar.activation(vv[:], vv[:], AF.Sqrt, bias=epsc[:])
                rs = rp.tile([128, 1], f32, tag="ers")
                nc.vector.reciprocal(rs[:], vv[:])
                xn = tp.tile([128, D], f32, tag="tc")
                nc.vector.tensor_scalar(xn[:], emb[:], scalar1=s1[:],
                                        scalar2=rs[:], op0=ALU.subtract,
                                        op1=ALU.mult)
                xg = tp.tile([128, D], f32, tag="tb")
                nc.vector.tensor_tensor(xg[:], xn[:], lnig_sb[:], op=ALU.mult)
                h0 = tp.tile([128, D], f32, tag="ta")
                nc.vector.tensor_tensor(h0[:], xg[:], lnib_sb[:], op=ALU.add)
                for d in range(ND):
                    ptr = ppm.tile([128, 128], f32, tag="mm")
                    nc.tensor.transpose(ptr[:], in_=h0[:, d * 128:(d + 1) * 128],
                                        identity=ident[:])
                    nc.vector.tensor_copy(h[d][:, tt * 128:(tt + 1) * 128], ptr[:])

            # ---------- layers ----------
            with (
                tc.tile_pool(name="wtm", bufs=2) as wtm,
                tc.tile_pool(name="wff", bufs=1) as wff,
                tc.tile_pool(name="wfo", bufs=3) as wfo,
            ):
                for l in range(0 if "layers" in ABL else L):
                    # ---- ln1 -> x ----
                    ln_fm(h, ln1g[l], ln1b[l], x)

                    # ---- r, k, v token-major; kv product ----
                    wr_sb = wtm.tile([128, ND * D], f32r, tag="wtm")
                    nc.sync.dma_start(
                        wr_sb[:].rearrange("p (a m) -> p a m", a=ND),
                        Wr[l][:].rearrange("(a p) m -> p a m", p=128))
                    r_sb = bg.tile([128, NT * D], f32, tag="r")
                    for tt in range(NT):
                        ps = ppm.tile([128, D], f32, tag="mm")
                        for kd in range(ND):
                            nc.tensor.matmul(
                                ps[:], lhsT=x[kd][:, tt * 128:(tt + 1) * 128],
                                rhs=wr_sb[:, kd * D:(kd + 1) * D],
                                start=(kd == 0), stop=(kd == ND - 1))
                        nc.scalar.activation(r_sb[:, tt * D:(tt + 1) * D], ps[:],
                                             AF.Sigmoid)
                    wk_sb = wtm.tile([128, ND * D], f32r, tag="wtm")
                    nc.sync.dma_start(
                        wk_sb[:].rearrange("p (a m) -> p a m", a=ND),
                        Wk[l][:].rearrange("(a p) m -> p a m", p=128))
                    k_sb = bg.tile([128, NT * D], f32, tag="ksb")
                    for tt in range(NT):
                        ps = ppm.tile([128, D], f32, tag="mm")
                        for kd in range(ND):
                            nc.tensor.matmul(
                                ps[:], lhsT=x[kd][:, tt * 128:(tt + 1) * 128],
                                rhs=wk_sb[:, kd * D:(kd + 1) * D],
                                start=(kd == 0), stop=(kd == ND - 1))
                        nc.scalar.activation(k_sb[:, tt * D:(tt + 1) * D], ps[:],
                                             AF.Copy)
                    wv_sb = wtm.tile([128, ND * D], f32r, tag="wtm")
                    nc.sync.dma_start(
                        wv_sb[:].rearrange("p (a m) -> p a m", a=ND),
                        Wv[l][:].rearrange("(a p) m -> p a m", p=128))
                    kv_sb = k_sb
                    for tt in range(NT):
                        ps = ppm.tile([128, D], f32, tag="mm")
                        for kd in range(ND):
                            nc.tensor.matmul(
                                ps[:], lhsT=x[kd][:, tt * 128:(tt + 1) * 128],
                                rhs=wv_sb[:, kd * D:(kd + 1) * D],
                                start=(kd == 0), stop=(kd == ND - 1))
                        nc.vector.tensor_tensor(kv_sb[:, tt * D:(tt + 1) * D],
                                                k_sb[:, tt * D:(tt + 1) * D],
                                                ps[:], op=ALU.mult)

                    # ---- decay scale factors (token-major, fp32 exact) ----
                    dec_sb = tp.tile([128, D], f32, tag="ta")
                    nc.sync.dma_start(dec_sb[:], decs[l][:])
                    nc.scalar.activation(dec_sb[:], dec_sb[:], AF.Sigmoid)
                    nc.vector.tensor_scalar_max(dec_sb[:], dec_sb[:], 1e-7)
                    ld_sb = tp.tile([128, D], f32, tag="ld")
                    nc.scalar.activation(ld_sb[:], dec_sb[:], AF.Ln)
                    scale_sb = bg.tile([128, NT * D], f32, tag="scale")
                    term_sb = bg.tile([128, NT * D], f32r, tag="term")
                    for tt in range(NT):
                        blk = slice(tt * D, (tt + 1) * D)
                        nc.vector.tensor_scalar_mul(scale_sb[:, blk], ld_sb[:],
                                                    tglob_sb[:, tt:tt + 1])
                        nc.scalar.activation(scale_sb[:, blk], scale_sb[:, blk],
                                             AF.Exp)
                        inv_t = tp.tile([128, D], f32, tag="tc")
                        nc.vector.tensor_scalar_max(inv_t[:], scale_sb[:, blk],
                                                    1e-10)
                        nc.vector.reciprocal(inv_t[:], inv_t[:])
                        nc.vector.tensor_tensor(term_sb[:, blk],
                                                kv_sb[:, blk], inv_t[:],
                                                op=ALU.mult)

                    # ---- cumsum over tokens (tri matmul) + cross-core carry ----
                    cum = [ppa.tile([128, D], f32, tag="acc", name=f"cum{l}_{i}")
                           for i in range(NT)]
                    for tt in range(NT):
                        for ss in range(tt):
                            nc.tensor.matmul(cum[tt][:], lhsT=ones_sq[:],
                                             rhs=term_sb[:, ss * D:(ss + 1) * D],
                                             start=(ss == 0), stop=False)
                        nc.tensor.matmul(cum[tt][:], lhsT=tri_sb[:],
                                         rhs=term_sb[:, tt * D:(tt + 1) * D],
                                         start=(tt == 0), stop=False)
                    ps_tot = pps.tile([1, D], f32, tag="st")
                    for ss in range(NT):
                        nc.tensor.matmul(ps_tot[:], lhsT=ones_col[:],
                                         rhs=term_sb[:, ss * D:(ss + 1) * D],
                                         start=(ss == 0), stop=(ss == NT - 1))
                    tot_r = rp.tile([1, D], f32, tag="totr")
                    nc.scalar.activation(tot_r[:], ps_tot[:], AF.Copy)
                    cin = dr.tile([1, D], f32, tag="cin")
                    nc.gpsimd.dma_start(cin[:], tot_r[:])
                    cout = dr.tile([GP, D], f32, tag="cout")
                    if "cc" not in ABL:
                        nc.gpsimd.collective_compute(
                            "AllGather", ALU.bypass, replica_groups=GROUPS4,
                            ins=[cin.opt()], outs=[cout.opt()])
                    gath = tp.tile([GP, D], f32, tag="gath")
                    nc.gpsimd.dma_start(gath[:], cout[:])
                    gath_r = tp.tile([GP, D], f32r, tag="gathr")
                    nc.vector.tensor_copy(gath_r[:], gath[:])
                    ps_c = pps.tile([1, D], f32, tag="st")
                    nc.tensor.matmul(ps_c[:], lhsT=gmask_sb[:], rhs=gath_r[:],
                                     start=True, stop=True)
                    carry_r = rp.tile([1, D], f32r, tag="carry")
                    nc.vector.tensor_copy(carry_r[:], ps_c[:])
                    for tt in range(NT):
                        nc.tensor.matmul(cum[tt][:], lhsT=one_row[:],
                                         rhs=carry_r[:], start=False, stop=True)

                    # ---- out = r * cum * scale;  GroupNorm stats ----
                    out_sb = scale_sb
                    rsums = []
                    qsums = []
                    for tt in range(NT):
                        blk = slice(tt * D, (tt + 1) * D)
                        o1 = tp.tile([128, D], f32, tag="ta")
                        nc.vector.tensor_tensor(o1[:], scale_sb[:, blk],
                                                cum[tt][:], op=ALU.mult)
                        nc.vector.tensor_tensor(out_sb[:, blk], o1[:],
                                                r_sb[:, blk], op=ALU.mult)
                        rs_ = rp.tile([128, 1], f32, tag=f"grs{tt}")
                        nc.vector.tensor_reduce(rs_[:], out_sb[:, blk],
                                                axis=mybir.AxisListType.X,
                                                op=ALU.add)
                        rsums.append(rs_)
                        sq_ = tp.tile([128, D], f32, tag="tb")
                        nc.gpsimd.tensor_tensor(sq_[:], out_sb[:, blk],
                                                out_sb[:, blk], op=ALU.mult)
                        qs_ = rp.tile([128, 1], f32, tag=f"gqs{tt}")
                        nc.vector.tensor_reduce(qs_[:], sq_[:],
                                                axis=mybir.AxisListType.X,
                                                op=ALU.add)
                        qsums.append(qs_)
                    for lst in (rsums, qsums):
                        nc.vector.tensor_tensor(lst[0][:], lst[0][:], lst[1][:],
                                                op=ALU.add)
                        nc.vector.tensor_tensor(lst[2][:], lst[2][:], lst[3][:],
                                                op=ALU.add)
                        nc.vector.tensor_tensor(lst[0][:], lst[0][:], lst[2][:],
                                                op=ALU.add)
                    rtot = rp.tile([128, 1], f32r, tag="rtot")
                    nc.vector.tensor_copy(rtot[:], rsums[0][:])
                    qtot = rp.tile([128, 1], f32r, tag="qtot")
                    nc.vector.tensor_copy(qtot[:], qsums[0][:])
                    ps_g = pps.tile([1, 2], f32, tag="st")
                    nc.tensor.matmul(ps_g[:], lhsT=rtot[:], rhs=ones_sq[:, 0:2],
                                     start=True, stop=True)
                    ps_g2 = pps.tile([1, 2], f32, tag="st")
                    nc.tensor.matmul(ps_g2[:], lhsT=qtot[:], rhs=ones_sq[:, 0:2],
                                     start=True, stop=True)
                    st_sb = rp.tile([1, 2], f32, tag="stsb")
                    nc.scalar.activation(st_sb[:, 0:1], ps_g[:, 0:1], AF.Copy)
                    nc.scalar.activation(st_sb[:, 1:2], ps_g2[:, 0:1], AF.Copy)
                    gn_in = dr.tile([1, 2], f32, tag="gnin")
                    nc.gpsimd.dma_start(gn_in[:], st_sb[:])
                    gn_out = dr.tile([GP, 2], f32, tag="gnout")
                    if "cc" not in ABL:
                        nc.gpsimd.collective_compute(
                            "AllGather", ALU.bypass, replica_groups=GROUPS4,
                            ins=[gn_in.opt()], outs=[gn_out.opt()])
                    gst = tp.tile([GP, 2], f32, tag="gst")
                    nc.gpsimd.dma_start(gst[:], gn_out[:])
                    gst_r = tp.tile([GP, 2], f32r, tag="gstr")
                    nc.vector.tensor_copy(gst_r[:], gst[:])
                    ps_gt = pps.tile([1, 2], f32, tag="st")
                    nc.tensor.matmul(ps_gt[:], lhsT=ones_col[:GP, :],
                                     rhs=gst_r[:], start=True, stop=True)
                    gs = rp.tile([1, 2], f32, tag="gs")
                    nc.scalar.activation(gs[:], ps_gt[:], AF.Copy)
                    NTOT = float(GP * TOK * D)
                    m_t = rp.tile([1, 1], f32, tag="mt")
                    nc.vector.tensor_scalar_mul(m_t[:], gs[:, 0:1], 1.0 / NTOT)
                    q_t = rp.tile([1, 1], f32, tag="qt")
                    nc.vector.tensor_scalar_mul(q_t[:], gs[:, 1:2], 1.0 / NTOT)
                    mm_ = rp.tile([1, 1], f32, tag="mm_")
                    nc.vector.tensor_tensor(mm_[:], m_t[:], m_t[:], op=ALU.mult)
                    nc.vector.tensor_tensor(q_t[:], q_t[:], mm_[:],
                                            op=ALU.subtract)
                    nc.scalar.activation(q_t[:], q_t[:], AF.Sqrt, bias=epsc[:1, :])
                    nc.vector.reciprocal(q_t[:], q_t[:])
                    mr2 = rp.tile([1, 2], f32r, tag="mr2")
                    nc.vector.tensor_copy(mr2[:, 0:1], m_t[:])
                    nc.vector.tensor_copy(mr2[:, 1:2], q_t[:])
                    ps_mv = pps.tile([128, 2], f32, tag="st")
                    nc.tensor.matmul(ps_mv[:], lhsT=one_row[:], rhs=mr2[:],
                                     start=True, stop=True)
                    m_col = rp.tile([128, 1], f32, tag="mcol")
                    nc.vector.tensor_copy(m_col[:], ps_mv[:, 0:1])
                    v_col = rp.tile([128, 1], f32, tag="vcol")
                    nc.vector.tensor_copy(v_col[:], ps_mv[:, 1:2])
                    gng_sb = bg.tile([128, D], f32, tag="gfull")
                    nc.sync.dma_start(gng_sb[:], gng[l][:])
                    gnb_sb = bg.tile([128, D], f32, tag="bfull")
                    nc.sync.dma_start(gnb_sb[:], gnb[l][:])
                    onrm_sb = bg.tile([128, NT * D], f32, tag="r")
                    for tt in range(NT):
                        blk = slice(tt * D, (tt + 1) * D)
                        g1 = tp.tile([128, D], f32, tag="ta")
                        nc.vector.tensor_scalar(g1[:], out_sb[:, blk],
                                                scalar1=m_col[:],
                                                scalar2=v_col[:],
                                                op0=ALU.subtract, op1=ALU.mult)
                        g2 = tp.tile([128, D], f32, tag="tb")
                        nc.vector.tensor_tensor(g2[:], g1[:], gng_sb[:],
                                                op=ALU.mult)
                        nc.vector.tensor_tensor(onrm_sb[:, blk], g2[:],
                                                gnb_sb[:], op=ALU.add)

                    # ---- transpose out_norm to feature-major; h += onrm @ Wo ----
                    onf = [bg.tile([128, TOK], f32r, tag=f"onf{d}", name=f"onf{l}_{d}")
                           for d in range(ND)]
                    for tt in range(NT):
                        for d in range(ND):
                            ptr = ppm.tile([128, 128], f32, tag="mm")
                            nc.tensor.transpose(
                                ptr[:],
                                in_=onrm_sb[:, tt * D + d * 128:tt * D + (d + 1) * 128],
                                identity=ident[:])
                            nc.vector.tensor_copy(
                                onf[d][:, tt * 128:(tt + 1) * 128], ptr[:])
                    wo_sb = wtm.tile([128, ND * D], f32r, tag="wtm")
                    nc.sync.dma_start(
                        wo_sb[:].rearrange("p (a m) -> p a m", a=ND),
                        Wo[l][:].rearrange("(a p) m -> p a m", p=128))
                    for d in range(ND):
                        ps = ppa.tile([128, TOK], f32, tag="acc")
                        for kd in range(ND):
                            nc.tensor.matmul(
                                ps[:],
                                lhsT=wo_sb[:, kd * D + d * 128:kd * D + (d + 1) * 128],
                                rhs=onf[kd][:], start=(kd == 0),
                                stop=(kd == ND - 1))
                        nc.vector.tensor_tensor(h[d][:], h[d][:], ps[:],
                                                op=ALU.add)

                    # ---- ln2 -> x2 (reuse x tiles); FFN ----
                    ln_fm(h, ln2g[l], ln2b[l], x)
                    FH = FF // 2
                    NFH = NF // 2
                    psf = [ppa.tile([128, TOK], f32, tag="acc", name=f"psf{l}_{i}")
                           for i in range(ND)]
                    for hf in range(2):
                        w1h = wff.tile([128, ND * FH], f32r, tag="w1h")
                        nc.sync.dma_start(
                            w1h[:].rearrange("p (a m) -> p a m", a=ND),
                            W1[l][:, hf * FH:(hf + 1) * FH].rearrange(
                                "(a p) m -> p a m", p=128))
                        w2h = wff.tile([128, ND * FH], f32r, tag="w2h")
                        nc.sync.dma_start(
                            w2h[:].rearrange("p (a m) -> p a m", a=ND),
                            W2[l][:, hf * FH:(hf + 1) * FH].rearrange(
                                "(a p) m -> p a m", p=128))
                        for fi in range(NFH):
                            ft = hf * NFH + fi
                            ps1 = ppm.tile([128, TOK], f32, tag="mm")
                            for kd in range(ND):
                                nc.tensor.matmul(
                                    ps1[:],
                                    lhsT=w1h[:, kd * FH + fi * 128:kd * FH + (fi + 1) * 128],
                                    rhs=x[kd][:], start=(kd == 0),
                                    stop=(kd == ND - 1))
                            ps2 = ppm.tile([128, TOK], f32, tag="mm")
                            for kd in range(ND):
                                nc.tensor.matmul(
                                    ps2[:],
                                    lhsT=w2h[:, kd * FH + fi * 128:kd * FH + (fi + 1) * 128],
                                    rhs=x[kd][:], start=(kd == 0),
                                    stop=(kd == ND - 1))
                            sl = tp.tile([128, TOK], f32, tag="ta")
                            nc.scalar.activation(sl[:], ps1[:], AF.Silu)
                            gt = tp.tile([128, TOK], f32r, tag="tr1")
                            nc.vector.tensor_tensor(gt[:], sl[:], ps2[:],
                                                    op=ALU.mult)
                            wf_t = wfo.tile([128, D], f32r, tag="wfo")
                            nc.sync.dma_start(
                                wf_t[:], Wf[l][ft * 128:(ft + 1) * 128, :])
                            for d in range(ND):
                                nc.tensor.matmul(
                                    psf[d][:],
                                    lhsT=wf_t[:, d * 128:(d + 1) * 128],
                                    rhs=gt[:], start=(ft == 0),
                                    stop=(ft == NF - 1))
                    for d in range(ND):
                        nc.vector.tensor_tensor(h[d][:], h[d][:], psf[d][:],
                                                op=ALU.add)

            # ---------- final LN + concepts + AllGather + logits ----------
            hd1cm = tc.tile_pool(name="head1", bufs=1)
            hd1 = hd1cm.__enter__()
            hd2cm = tc.tile_pool(name="head2", bufs=2)
            hd2 = hd2cm.__enter__()
            xo = [bg.tile([128, TOK], f32r, tag=f"x{d}", name=f"xo{d}") for d in range(ND)]
            ln_fm(h, lnog, lnob, xo)
            encw_sb = cp.tile([128, ND * CD], f32r)
            nc.sync.dma_start(
                encw_sb[:].rearrange("p (a m) -> p a m", a=ND),
                encW[:].rearrange("(a p) m -> p a m", p=128))
            encb_sb = cp.tile([CD, 1], f32)
            nc.sync.dma_start(encb_sb[:], encB[:])
            ps_cc = ppm.tile([CD, TOK], f32, tag="mm")
            for kd in range(ND):
                nc.tensor.matmul(ps_cc[:],
                                 lhsT=encw_sb[:, kd * CD:(kd + 1) * CD],
                                 rhs=xo[kd][:], start=(kd == 0),
                                 stop=(kd == ND - 1))
            conc_l = tp.tile([CD, TOK], f32, tag="ta")
            nc.scalar.activation(conc_l[:], ps_cc[:], AF.Identity, bias=encb_sb[:])
            cc_in = dr.tile([CD, TOK], f32, tag="ccin")
            nc.gpsimd.dma_start(cc_in[:], conc_l[:])
            cc_out = dr.tile([NC * CD, TOK], f32, tag="ccout")
            if "cc" not in ABL:
                nc.gpsimd.collective_compute(
                    "AllGather", ALU.bypass, replica_groups=GROUPS8,
                    ins=[cc_in.opt()], outs=[cc_out.opt()])
            conc_x = hd1.tile([CD + 1, B * T], f32r, tag="concx")
            for c in range(NC):
                stg = tp.tile([CD, TOK], f32, tag="ta")
                nc.sync.dma_start(stg[:], cc_out[c * CD:(c + 1) * CD, :])
                nc.vector.tensor_copy(conc_x[:CD, c * TOK:(c + 1) * TOK],
                                      stg[:])
            nc.sync.dma_start(conc_x[CD:CD + 1, :], onesrow4k[:])
            dec_sb2 = hd1.tile([CD + 1, VS], f32r, tag="decsb")
            nc.sync.dma_start(dec_sb2[:CD, :], decW[:])
            nc.sync.dma_start(dec_sb2[CD:CD + 1, :], decB[:])
            for tt in range(0 if "head" in ABL else NTT):
                lst = hd2.tile([128, VS], f32, tag="lst")
                for vc in range(NVC):
                    ps = ppa.tile([128, VCW], f32, tag="acc")
                    nc.tensor.matmul(
                        ps[:], lhsT=conc_x[:, tt * 128:(tt + 1) * 128],
                        rhs=dec_sb2[:, vc * VCW:(vc + 1) * VCW],
                        start=True, stop=True)
                    if vc % 2 == 0:
                        nc.vector.tensor_copy(lst[:, vc * VCW:(vc + 1) * VCW],
                                              ps[:])
                    else:
                        nc.scalar.activation(lst[:, vc * VCW:(vc + 1) * VCW],
                                             ps[:], AF.Copy)
                nc.sync.dma_start(logits[tt * 128:(tt + 1) * 128, :], lst[:])
            hd2cm.__exit__(None, None, None)
            hd1cm.__exit__(None, None, None)

    nc.finalize()
    return nc


_NC_CACHE = None


def _get_nc():
    global _NC_CACHE
    if _NC_CACHE is None:
        _NC_CACHE = _build()
    return _NC_CACHE


def kernel(**inputs):
    f = np.float32
    idx = np.asarray(inputs["idx"])
    embed = np.asarray(inputs["embed"], f)
    tri = np.triu(np.ones((128, 128), f))
    base = {
        "embed": embed,
        "tri": tri,
        "onescol": np.ones((128, 1), f),
        "onerow": np.ones((1, 128), f),
        "onessq": np.ones((128, 128), f),
        "onesrow4k": np.ones((1, B * T), f),
        "lnig": np.broadcast_to(np.asarray(inputs["ln_in_g"], f)[None, :],
                                (128, D)).copy(),
        "lnib": np.broadcast_to(np.asarray(inputs["ln_in_b"], f)[None, :],
                                (128, D)).copy(),
        "lnog": np.asarray(inputs["ln_out_g"], f).reshape(D, 1),
        "lnob": np.asarray(inputs["ln_out_b"], f).reshape(D, 1),
        "encW": np.asarray(inputs["enc_W"], f),
        "encB": np.asarray(inputs["enc_b"], f).reshape(CD, 1),
    }
    for l in range(L):
        base[f"Wr{l}"] = np.asarray(inputs["Wr"][l], f)
        base[f"Wk{l}"] = np.asarray(inputs["Wk"][l], f)
        base[f"Wv{l}"] = np.asarray(inputs["Wv"][l], f)
        base[f"Wo{l}"] = np.asarray(inputs["Wo_tm"][l], f)
        base[f"W1{l}"] = np.asarray(inputs["W1"][l], f)
        base[f"W2{l}"] = np.asarray(inputs["W2"][l], f)
        base[f"Wf{l}"] = np.asarray(inputs["Wo_ff"][l], f)
        base[f"ln1g{l}"] = np.asarray(inputs["ln1_g"][l], f).reshape(D, 1)
        base[f"ln1b{l}"] = np.asarray(inputs["ln1_b"][l], f).reshape(D, 1)
        base[f"ln2g{l}"] = np.asarray(inputs["ln2_g"][l], f).reshape(D, 1)
        base[f"ln2b{l}"] = np.asarray(inputs["ln2_b"][l], f).reshape(D, 1)
        base[f"gng{l}"] = np.broadcast_to(
            np.asarray(inputs["gn_g"][l], f)[None, :], (128, D)).copy()
        base[f"gnb{l}"] = np.broadcast_to(
            np.asarray(inputs["gn_b"][l], f)[None, :], (128, D)).copy()
        base[f"dec{l}"] = np.broadcast_to(
            np.asarray(inputs["decay"][l], f)[None, :], (128, D)).copy()
    dec_W = np.asarray(inputs["dec_W"], f)
    dec_b = np.asarray(inputs["dec_b"], f)

    in_maps = []
    for c in range(NC):
        b = c // GP
        r = c % GP
        s = r * TOK
        m = dict(base)
        m["idxs"] = idx[b, s:s + TOK].astype(np.int32).reshape(TOK, 1)
        m["tglob"] = (s + np.arange(TOK, dtype=f)).reshape(NT, 128).T.copy()
        m["gmask"] = (np.arange(GP) < r).astype(f).reshape(GP, 1)
        m["decW"] = dec_W[:, c * VS:(c + 1) * VS].copy()
        m["decB"] = dec_b[c * VS:(c + 1) * VS].reshape(1, VS).copy()
        in_maps.append(m)

    nc = _get_nc()
    res = run_bass_kernel_spmd(nc, in_maps, core_ids=list(range(NC)))
    if res.exec_time_ns is not None:
        print(f"HW exec time: {res.exec_time_ns} ns", flush=True)
    out = np.concatenate([res.results[c]["logits"] for c in range(NC)], axis=1)
    return out.reshape(B, T, V).astype(np.float32)


# revision 29
# speedup vs baseline: 1.0547x; 1.0547x over previous
"""ConceptRWKV Trainium2 kernel (8 NeuronCores, SPMD).

Sharding:
  - Trunk: sequence-parallel. Core c handles batch c//4, token chunk
    (c%4)*512. Residual h kept feature-major [D(4x128 part-tiles), 512 tok]
    in SBUF float32r. Per-layer cross-core exchange: decay-cumsum carry
    (subgroup AllGather of local totals + mask matmul) and GroupNorm
    global stats (subgroup AllReduce).
  - Head: V-sharded. concepts [64, 4096] AllGathered across all 8 cores;
    each core computes logits for its own 4000-column V slice.
Host reassembles logits by concatenating V slices.
"""

import numpy as np

import concourse.bass as bass
import concourse.mybir as mybir
import concourse.tile as tile
from concourse import bacc
from concourse.bass_utils import run_bass_kernel_spmd
from concourse.masks import make_identity

B, T, V, D, FF, L, CD = 2, 2048, 32000, 512, 2048, 6, 64
EPS = 1e-5
NC = 8
GP = 4          # cores per batch group
TOK = 512       # tokens per core
NT = TOK // 128  # token tiles per core
ND = D // 128    # d tiles
NF = FF // 128   # ff tiles
VS = V // NC     # vocab slice per core
NVC = 8
VCW = VS // NVC  # 500, one psum bank
NTT = (B * T) // 128  # 32 token tiles over the full stream

f32 = mybir.dt.float32
f32r = mybir.dt.float32r
i32 = mybir.dt.int32
AF = mybir.ActivationFunctionType
ALU = mybir.AluOpType

GROUPS4 = [[0, 1, 2, 3], [4, 5, 6, 7]]
GROUPS8 = [list(range(NC))]


def _build():
    import os
    ABL = set(os.environ.get("RWKV_ABLATE", "").split(","))
    nc = bacc.Bacc("TRN2", target_bir_lowering=False, debug=False,
                   num_devices=NC)

    # ---- parameters ----
    embed = nc.declare_dram_parameter("embed", [V, D], f32, isOutput=False)
    idxs = nc.declare_dram_parameter("idxs", [TOK, 1], i32, isOutput=False)
    tglob = nc.declare_dram_parameter("tglob", [128, NT], f32, isOutput=False)
    gmask = nc.declare_dram_parameter("gmask", [GP, 1], f32r, isOutput=False)
    tri = nc.declare_dram_parameter("tri", [128, 128], f32r, isOutput=False)
    onescol = nc.declare_dram_parameter("onescol", [128, 1], f32r, isOutput=False)
    onerow = nc.declare_dram_parameter("onerow", [1, 128], f32r, isOutput=False)
    onessq = nc.declare_dram_parameter("onessq", [128, 128], f32r, isOutput=False)
    onesrow4k = nc.declare_dram_parameter("onesrow4k", [1, B * T], f32r,
                                          isOutput=False)
    lnig = nc.declare_dram_parameter("lnig", [128, D], f32, isOutput=False)
    lnib = nc.declare_dram_parameter("lnib", [128, D], f32, isOutput=False)
    lnog = nc.declare_dram_parameter("lnog", [D, 1], f32, isOutput=False)
    lnob = nc.declare_dram_parameter("lnob", [D, 1], f32, isOutput=False)
    encW = nc.declare_dram_parameter("encW", [D, CD], f32r, isOutput=False)
    encB = nc.declare_dram_parameter("encB", [CD, 1], f32, isOutput=False)
    decW = nc.declare_dram_parameter("decW", [CD, VS], f32r, isOutput=False)
    decB = nc.declare_dram_parameter("decB", [1, VS], f32r, isOutput=False)

    Wr, Wk, Wv, Wo, W1, W2, Wf = [], [], [], [], [], [], []
    ln1g, ln1b, ln2g, ln2b, gng, gnb, decs = [], [], [], [], [], [], []
    for l in range(L):
        Wr.append(nc.declare_dram_parameter(f"Wr{l}", [D, D], f32r, isOutput=False))
        Wk.append(nc.declare_dram_parameter(f"Wk{l}", [D, D], f32r, isOutput=False))
        Wv.append(nc.declare_dram_parameter(f"Wv{l}", [D, D], f32r, isOutput=False))
        Wo.append(nc.declare_dram_parameter(f"Wo{l}", [D, D], f32r, isOutput=False))
        W1.append(nc.declare_dram_parameter(f"W1{l}", [D, FF], f32r, isOutput=False))
        W2.append(nc.declare_dram_parameter(f"W2{l}", [D, FF], f32r, isOutput=False))
        Wf.append(nc.declare_dram_parameter(f"Wf{l}", [FF, D], f32r, isOutput=False))
        ln1g.append(nc.declare_dram_parameter(f"ln1g{l}", [D, 1], f32, isOutput=False))
        ln1b.append(nc.declare_dram_parameter(f"ln1b{l}", [D, 1], f32, isOutput=False))
        ln2g.append(nc.declare_dram_parameter(f"ln2g{l}", [D, 1], f32, isOutput=False))
        ln2b.append(nc.declare_dram_parameter(f"ln2b{l}", [D, 1], f32, isOutput=False))
        gng.append(nc.declare_dram_parameter(f"gng{l}", [128, D], f32, isOutput=False))
        gnb.append(nc.declare_dram_parameter(f"gnb{l}", [128, D], f32, isOutput=False))
        decs.append(nc.declare_dram_parameter(f"dec{l}", [128, D], f32, isOutput=False))

    logits = nc.declare_dram_parameter("logits", [B * T, VS], f32, isOutput=True)

    with tile.TileContext(nc) as tc:
        with (
            tc.tile_pool(name="const", bufs=1) as cp,
            tc.tile_pool(name="persist", bufs=1) as hp,
            tc.tile_pool(name="big", bufs=1) as bg,
            tc.tile_pool(name="tmp", bufs=2) as tp,
            tc.tile_pool(name="rows", bufs=1) as rp,
            tc.tile_pool(name="ppmm", bufs=2, space="PSUM") as ppm,
            tc.tile_pool(name="ppacc", bufs=4, space="PSUM") as ppa,
            tc.tile_pool(name="ppst", bufs=2, space="PSUM") as pps,
            tc.tile_pool(name="dram", bufs=1, space="DRAM") as dr,
        ):
            # ---- constants ----
            ident = cp.tile([128, 128], f32)
            make_identity(nc, ident[:])
            tri_sb = cp.tile([128, 128], f32r)
            nc.sync.dma_start(tri_sb[:], tri[:])
            ones_sq = cp.tile([128, 128], f32r)
            nc.sync.dma_start(ones_sq[:], onessq[:])
            ones_col = cp.tile([128, 1], f32r)
            nc.sync.dma_start(ones_col[:], onescol[:])
            one_row = cp.tile([1, 128], f32r)
            nc.sync.dma_start(one_row[:], onerow[:])
            gmask_sb = cp.tile([GP, 1], f32r)
            nc.sync.dma_start(gmask_sb[:], gmask[:])
            tglob_sb = cp.tile([128, NT], f32)
            nc.sync.dma_start(tglob_sb[:], tglob[:])
            epsc = cp.tile([128, 1], f32)
            nc.vector.memset(epsc[:], EPS)

            # persistent residual stream, feature-major f32r
            h = [hp.tile([128, TOK], f32r, tag=f"h{d}", name=f"h{d}") for d in range(ND)]

            # ---------- helpers ----------
            def ln_fm(hin, g_dram, b_dram, out_tiles):
                """LayerNorm over D (partitions) of feature-major hin -> f32r."""
                # per-token sum / sumsq via ones-matmuls
                ps_s = pps.tile([1, TOK], f32, tag="st")
                ps_q = pps.tile([1, TOK], f32, tag="st")
                for d in range(ND):
                    nc.tensor.matmul(ps_s[:], lhsT=ones_col[:], rhs=hin[d][:],
                                     start=(d == 0), stop=(d == ND - 1))
                for d in range(ND):
                    hsq = tp.tile([128, TOK], f32r, tag="tr1")
                    nc.gpsimd.tensor_tensor(hsq[:], hin[d][:], hin[d][:],
                                             op=ALU.mult)
                    nc.tensor.matmul(ps_q[:], lhsT=ones_col[:], rhs=hsq[:],
                                     start=(d == 0), stop=(d == ND - 1))
                mean_r = rp.tile([1, TOK], f32, tag="mean")
                nc.vector.tensor_scalar_mul(mean_r[:], ps_s[:], 1.0 / D)
                msq_r = rp.tile([1, TOK], f32, tag="msq")
                nc.vector.tensor_scalar_mul(msq_r[:], ps_q[:], 1.0 / D)
                m2_r = rp.tile([1, TOK], f32, tag="m2")
                nc.vector.tensor_tensor(m2_r[:], mean_r[:], mean_r[:], op=ALU.mult)
                var_r = rp.tile([1, TOK], f32, tag="var")
                nc.vector.tensor_tensor(var_r[:], msq_r[:], m2_r[:],
                                        op=ALU.subtract)
                nc.scalar.activation(var_r[:], var_r[:], AF.Sqrt, bias=epsc[:1, :])
                rstd_r = rp.tile([1, TOK], f32, tag="rstd")
                nc.vector.reciprocal(rstd_r[:], var_r[:])
                # A = rstd, Bc = -mean*rstd   (f32r rows for K=1 bcast matmul)
                a_r = rp.tile([1, TOK], f32r, tag="arow")
                nc.vector.tensor_copy(a_r[:], rstd_r[:])
                b_t = rp.tile([1, TOK], f32, tag="btmp")
                nc.vector.tensor_tensor(b_t[:], mean_r[:], rstd_r[:], op=ALU.mult)
                b_r = rp.tile([1, TOK], f32r, tag="brow")
                nc.vector.tensor_scalar_mul(b_r[:], b_t[:], -1.0)
                ps_a = pps.tile([128, TOK], f32, tag="st")
                nc.tensor.matmul(ps_a[:], lhsT=one_row[:], rhs=a_r[:],
                                 start=True, stop=True)
                ps_b = pps.tile([128, TOK], f32, tag="st")
                nc.tensor.matmul(ps_b[:], lhsT=one_row[:], rhs=b_r[:],
                                 start=True, stop=True)
                for d in range(ND):
                    gc = rp.tile([128, 1], f32, tag="gcol")
                    nc.sync.dma_start(gc[:], g_dram[d * 128:(d + 1) * 128, :])
                    bc = rp.tile([128, 1], f32, tag="bcol")
                    nc.sync.dma_start(bc[:], b_dram[d * 128:(d + 1) * 128, :])
                    t1 = tp.tile([128, TOK], f32, tag="ta")
                    nc.vector.tensor_tensor(t1[:], hin[d][:], ps_a[:], op=ALU.mult)
                    t2 = tp.tile([128, TOK], f32, tag="tb")
                    nc.vector.tensor_tensor(t2[:], t1[:], ps_b[:], op=ALU.add)
                    nc.scalar.activation(out_tiles[d][:], t2[:], AF.Identity,
                                         bias=bc[:], scale=gc[:])

            # ---------- embedding + input LN ----------
            x = [bg.tile([128, TOK], f32r, tag=f"x{d}", name=f"x{d}") for d in range(ND)]
            lnig_sb = bg.tile([128, D], f32, tag="gfull")
            nc.sync.dma_start(lnig_sb[:], lnig[:])
            lnib_sb = bg.tile([128, D], f32, tag="bfull")
            nc.sync.dma_start(lnib_sb[:], lnib[:])
            for tt in range(NT):
                idx_t = tp.tile([128, 1], i32, tag="idxt")
                nc.sync.dma_start(idx_t[:], idxs[tt * 128:(tt + 1) * 128, :])
                emb = tp.tile([128, D], f32, tag="ta")
                nc.gpsimd.indirect_dma_start(
                    out=emb[:], out_offset=None, in_=embed[:],
                    in_offset=bass.IndirectOffsetOnAxis(ap=idx_t[:, :1], axis=0))
                s1 = rp.tile([128, 1], f32, tag="es1")
                nc.vector.tensor_reduce(s1[:], emb[:], axis=mybir.AxisListType.X,
                                        op=ALU.add)
                nc.vector.tensor_scalar_mul(s1[:], s1[:], 1.0 / D)
                sq = tp.tile([128, D], f32, tag="tb")
                nc.vector.tensor_tensor(sq[:], emb[:], emb[:], op=ALU.mult)
                s2 = rp.tile([128, 1], f32, tag="es2")
                nc.vector.tensor_reduce(s2[:], sq[:], axis=mybir.AxisListType.X,
                                        op=ALU.add)
                nc.vector.tensor_scalar_mul(s2[:], s2[:], 1.0 / D)
                m2 = rp.tile([128, 1], f32, tag="em2")
                nc.vector.tensor_tensor(m2[:], s1[:], s1[:], op=ALU.mult)
                vv = rp.tile([128, 1], f32, tag="evv")
                nc.vector.tensor_tensor(vv[:], s2[:], m2[:], op=ALU.subtract)
                nc.scalar.activation(vv[:], vv[:], AF.Sqrt, bias=epsc[:])
                rs = rp.tile([128, 1], f32, tag="ers")
                nc.vector.reciprocal(rs[:], vv[:])
                xn = tp.tile([128, D], f32, tag="tc")
                nc.vector.tensor_scalar(xn[:], emb[:], scalar1=s1[:],
                                        scalar2=rs[:], op0=ALU.subtract,
                                        op1=ALU.mult)
                xg = tp.tile([128, D], f32, tag="tb")
                nc.vector.tensor_tensor(xg[:], xn[:], lnig_sb[:], op=ALU.mult)
                h0 = tp.tile([128, D], f32, tag="ta")
                nc.vector.tensor_tensor(h0[:], xg[:], lnib_sb[:], op=ALU.add)
                for d in range(ND):
                    ptr = ppm.tile([128, 128], f32, tag="mm")
                    nc.tensor.transpose(ptr[:], in_=h0[:, d * 128:(d + 1) * 128],
                                        identity=ident[:])
                    nc.vector.tensor_copy(h[d][:, tt * 128:(tt + 1) * 128], ptr[:])

            # ---------- layers ----------
            with (
                tc.tile_pool(name="wtm", bufs=2) as wtm,
                tc.tile_pool(name="wff", bufs=2) as wff,
                tc.tile_pool(name="wfo", bufs=3) as wfo,
            ):
                for l in range(0 if "layers" in ABL else L):
                    # ---- ln1 -> x ----
                    ln_fm(h, ln1g[l], ln1b[l], x)

                    # ---- r, k, v token-major; kv product ----
                    wr_sb = wtm.tile([128, ND * D], f32r, tag="wtm")
                    nc.sync.dma_start(
                        wr_sb[:].rearrange("p (a m) -> p a m", a=ND),
                        Wr[l][:].rearrange("(a p) m -> p a m", p=128))
                    r_sb = bg.tile([128, NT * D], f32, tag="r")
                    for tt in range(NT):
                        ps = ppm.tile([128, D], f32, tag="mm")
                        for kd in range(ND):
                            nc.tensor.matmul(
                                ps[:], lhsT=x[kd][:, tt * 128:(tt + 1) * 128],
                                rhs=wr_sb[:, kd * D:(kd + 1) * D],
                                start=(kd == 0), stop=(kd == ND - 1))
                        nc.scalar.activation(r_sb[:, tt * D:(tt + 1) * D], ps[:],
                                             AF.Sigmoid)
                    wk_sb = wtm.tile([128, ND * D], f32r, tag="wtm")
                    nc.sync.dma_start(
                        wk_sb[:].rearrange("p (a m) -> p a m", a=ND),
                        Wk[l][:].rearrange("(a p) m -> p a m", p=128))
                    k_sb = bg.tile([128, NT * D], f32, tag="ksb")
                    for tt in range(NT):
                        ps = ppm.tile([128, D], f32, tag="mm")
                        for kd in range(ND):
                            nc.tensor.matmul(
                                ps[:], lhsT=x[kd][:, tt * 128:(tt + 1) * 128],
                                rhs=wk_sb[:, kd * D:(kd + 1) * D],
                                start=(kd == 0), stop=(kd == ND - 1))
                        nc.scalar.activation(k_sb[:, tt * D:(tt + 1) * D], ps[:],
                                             AF.Copy)
                    wv_sb = wtm.tile([128, ND * D], f32r, tag="wtm")
                    nc.sync.dma_start(
                        wv_sb[:].rearrange("p (a m) -> p a m", a=ND),
                        Wv[l][:].rearrange("(a p) m -> p a m", p=128))
                    kv_sb = k_sb
                    for tt in range(NT):
                        ps = ppm.tile([128, D], f32, tag="mm")
                        for kd in range(ND):
                            nc.tensor.matmul(
                                ps[:], lhsT=x[kd][:, tt * 128:(tt + 1) * 128],
                                rhs=wv_sb[:, kd * D:(kd + 1) * D],
                                start=(kd == 0), stop=(kd == ND - 1))
                        nc.vector.tensor_tensor(kv_sb[:, tt * D:(tt + 1) * D],
                                                k_sb[:, tt * D:(tt + 1) * D],
                                                ps[:], op=ALU.mult)

                    # ---- decay scale factors (token-major, fp32 exact) ----
                    dec_sb = tp.tile([128, D], f32, tag="ta")
                    nc.sync.dma_start(dec_sb[:], decs[l][:])
                    nc.scalar.activation(dec_sb[:], dec_sb[:], AF.Sigmoid)
                    nc.vector.tensor_scalar_max(dec_sb[:], dec_sb[:], 1e-7)
                    ld_sb = tp.tile([128, D], f32, tag="ld")
                    nc.scalar.activation(ld_sb[:], dec_sb[:], AF.Ln)
                    scale_sb = bg.tile([128, NT * D], f32, tag="scale")
                    term_sb = bg.tile([128, NT * D], f32r, tag="term")
                    for tt in range(NT):
                        blk = slice(tt * D, (tt + 1) * D)
                        nc.vector.tensor_scalar_mul(scale_sb[:, blk], ld_sb[:],
                                                    tglob_sb[:, tt:tt + 1])
                        nc.scalar.activation(scale_sb[:, blk], scale_sb[:, blk],
                                             AF.Exp)
                        inv_t = tp.tile([128, D], f32, tag="tc")
                        nc.vector.tensor_scalar_max(inv_t[:], scale_sb[:, blk],
                                                    1e-10)
                        nc.vector.reciprocal(inv_t[:], inv_t[:])
                        nc.vector.tensor_tensor(term_sb[:, blk],
                                                kv_sb[:, blk], inv_t[:],
                                                op=ALU.mult)

                    # ---- cumsum over tokens (tri matmul) + cross-core carry ----
                    cum = [ppa.tile([128, D], f32, tag="acc", name=f"cum{l}_{i}")
                           for i in range(NT)]
                    for tt in range(NT):
                        for ss in range(tt):
                            nc.tensor.matmul(cum[tt][:], lhsT=ones_sq[:],
                                             rhs=term_sb[:, ss * D:(ss + 1) * D],
                                             start=(ss == 0), stop=False)
                        nc.tensor.matmul(cum[tt][:], lhsT=tri_sb[:],
                                         rhs=term_sb[:, tt * D:(tt + 1) * D],
                                         start=(tt == 0), stop=False)
                    ps_tot = pps.tile([1, D], f32, tag="st")
                    for ss in range(NT):
                        nc.tensor.matmul(ps_tot[:], lhsT=ones_col[:],
                                         rhs=term_sb[:, ss * D:(ss + 1) * D],
                                         start=(ss == 0), stop=(ss == NT - 1))
                    tot_r = rp.tile([1, D], f32, tag="totr")
                    nc.scalar.activation(tot_r[:], ps_tot[:], AF.Copy)
                    cin = dr.tile([1, D], f32, tag="cin")
                    nc.gpsimd.dma_start(cin[:], tot_r[:])
                    cout = dr.tile([GP, D], f32, tag="cout")
                    if "cc" not in ABL:
                        nc.gpsimd.collective_compute(
                            "AllGather", ALU.bypass, replica_groups=GROUPS4,
                            ins=[cin.opt()], outs=[cout.opt()])
                    gath = tp.tile([GP, D], f32, tag="gath")
                    nc.gpsimd.dma_start(gath[:], cout[:])
                    gath_r = tp.tile([GP, D], f32r, tag="gathr")
                    nc.vector.tensor_copy(gath_r[:], gath[:])
                    ps_c = pps.tile([1, D], f32, tag="st")
                    nc.tensor.matmul(ps_c[:], lhsT=gmask_sb[:], rhs=gath_r[:],
                                     start=True, stop=True)
                    carry_r = rp.tile([1, D], f32r, tag="carry")
                    nc.vector.tensor_copy(carry_r[:], ps_c[:])
                    for tt in range(NT):
                        nc.tensor.matmul(cum[tt][:], lhsT=one_row[:],
                                         rhs=carry_r[:], start=False, stop=True)

                    # ---- out = r * cum * scale;  GroupNorm stats ----
                    out_sb = scale_sb
                    rsums = []
                    qsums = []
                    for tt in range(NT):
                        blk = slice(tt * D, (tt + 1) * D)
                        o1 = tp.tile([128, D], f32, tag="ta")
                        nc.vector.tensor_tensor(o1[:], scale_sb[:, blk],
                                                cum[tt][:], op=ALU.mult)
                        nc.vector.tensor_tensor(out_sb[:, blk], o1[:],
                                                r_sb[:, blk], op=ALU.mult)
                        rs_ = rp.tile([128, 1], f32, tag=f"grs{tt}")
                        nc.vector.tensor_reduce(rs_[:], out_sb[:, blk],
                                                axis=mybir.AxisListType.X,
                                                op=ALU.add)
                        rsums.append(rs_)
                        sq_ = tp.tile([128, D], f32, tag="tb")
                        nc.gpsimd.tensor_tensor(sq_[:], out_sb[:, blk],
                                                out_sb[:, blk], op=ALU.mult)
                        qs_ = rp.tile([128, 1], f32, tag=f"gqs{tt}")
                        nc.vector.tensor_reduce(qs_[:], sq_[:],
                                                axis=mybir.AxisListType.X,
                                                op=ALU.add)
                        qsums.append(qs_)
                    for lst in (rsums, qsums):
                        nc.vector.tensor_tensor(lst[0][:], lst[0][:], lst[1][:],
                                                op=ALU.add)
                        nc.vector.tensor_tensor(lst[2][:], lst[2][:], lst[3][:],
                                                op=ALU.add)
                        nc.vector.tensor_tensor(lst[0][:], lst[0][:], lst[2][:],
                                                op=ALU.add)
                    rtot = rp.tile([128, 1], f32r, tag="rtot")
                    nc.vector.tensor_copy(rtot[:], rsums[0][:])
                    qtot = rp.tile([128, 1], f32r, tag="qtot")
                    nc.vector.tensor_copy(qtot[:], qsums[0][:])
                    ps_g = pps.tile([1, 2], f32, tag="st")
                    nc.tensor.matmul(ps_g[:], lhsT=rtot[:], rhs=ones_sq[:, 0:2],
                                     start=True, stop=True)
                    ps_g2 = pps.tile([1, 2], f32, tag="st")
                    nc.tensor.matmul(ps_g2[:], lhsT=qtot[:], rhs=ones_sq[:, 0:2],
                                     start=True, stop=True)
                    st_sb = rp.tile([1, 2], f32, tag="stsb")
                    nc.scalar.activation(st_sb[:, 0:1], ps_g[:, 0:1], AF.Copy)
                    nc.scalar.activation(st_sb[:, 1:2], ps_g2[:, 0:1], AF.Copy)
                    gn_in = dr.tile([1, 2], f32, tag="gnin")
                    nc.gpsimd.dma_start(gn_in[:], st_sb[:])
                    gn_out = dr.tile([GP, 2], f32, tag="gnout")
                    if "cc" not in ABL:
                        nc.gpsimd.collective_compute(
                            "AllGather", ALU.bypass, replica_groups=GROUPS4,
                            ins=[gn_in.opt()], outs=[gn_out.opt()])
                    gst = tp.tile([GP, 2], f32, tag="gst")
                    nc.gpsimd.dma_start(gst[:], gn_out[:])
                    gst_r = tp.tile([GP, 2], f32r, tag="gstr")
                    nc.vector.tensor_copy(gst_r[:], gst[:])
                    ps_gt = pps.tile([1, 2], f32, tag="st")
                    nc.tensor.matmul(ps_gt[:], lhsT=ones_col[:GP, :],
                                     rhs=gst_r[:], start=True, stop=True)
                    gs = rp.tile([1, 2], f32, tag="gs")
                    nc.scalar.activation(gs[:], ps_gt[:], AF.Copy)
                    NTOT = float(GP * TOK * D)
                    m_t = rp.tile([1, 1], f32, tag="mt")
                    nc.vector.tensor_scalar_mul(m_t[:], gs[:, 0:1], 1.0 / NTOT)
                    q_t = rp.tile([1, 1], f32, tag="qt")
                    nc.vector.tensor_scalar_mul(q_t[:], gs[:, 1:2], 1.0 / NTOT)
                    mm_ = rp.tile([1, 1], f32, tag="mm_")
                    nc.vector.tensor_tensor(mm_[:], m_t[:], m_t[:], op=ALU.mult)
                    nc.vector.tensor_tensor(q_t[:], q_t[:], mm_[:],
                                            op=ALU.subtract)
                    nc.scalar.activation(q_t[:], q_t[:], AF.Sqrt, bias=epsc[:1, :])
                    nc.vector.reciprocal(q_t[:], q_t[:])
                    mr2 = rp.tile([1, 2], f32r, tag="mr2")
                    nc.vector.tensor_copy(mr2[:, 0:1], m_t[:])
                    nc.vector.tensor_copy(mr2[:, 1:2], q_t[:])
                    ps_mv = pps.tile([128, 2], f32, tag="st")
                    nc.tensor.matmul(ps_mv[:], lhsT=one_row[:], rhs=mr2[:],
                                     start=True, stop=True)
                    m_col = rp.tile([128, 1], f32, tag="mcol")
                    nc.vector.tensor_copy(m_col[:], ps_mv[:, 0:1])
                    v_col = rp.tile([128, 1], f32, tag="vcol")
                    nc.vector.tensor_copy(v_col[:], ps_mv[:, 1:2])
                    gng_sb = bg.tile([128, D], f32, tag="gfull")
                    nc.sync.dma_start(gng_sb[:], gng[l][:])
                    gnb_sb = bg.tile([128, D], f32, tag="bfull")
                    nc.sync.dma_start(gnb_sb[:], gnb[l][:])
                    onrm_sb = bg.tile([128, NT * D], f32, tag="r")
                    for tt in range(NT):
                        blk = slice(tt * D, (tt + 1) * D)
                        g1 = tp.tile([128, D], f32, tag="ta")
                        nc.vector.tensor_scalar(g1[:], out_sb[:, blk],
                                                scalar1=m_col[:],
                                                scalar2=v_col[:],
                                                op0=ALU.subtract, op1=ALU.mult)
                        g2 = tp.tile([128, D], f32, tag="tb")
                        nc.vector.tensor_tensor(g2[:], g1[:], gng_sb[:],
                                                op=ALU.mult)
                        nc.vector.tensor_tensor(onrm_sb[:, blk], g2[:],
                                                gnb_sb[:], op=ALU.add)

                    # ---- transpose out_norm to feature-major; h += onrm @ Wo ----
                    onf = [bg.tile([128, TOK], f32r, tag=f"onf{d}", name=f"onf{l}_{d}")
                           for d in range(ND)]
                    for tt in range(NT):
                        for d in range(ND):
                            ptr = ppm.tile([128, 128], f32, tag="mm")
                            nc.tensor.transpose(
                                ptr[:],
                                in_=onrm_sb[:, tt * D + d * 128:tt * D + (d + 1) * 128],
                                identity=ident[:])
                            nc.vector.tensor_copy(
                                onf[d][:, tt * 128:(tt + 1) * 128], ptr[:])
                    wo_sb = wtm.tile([128, ND * D], f32r, tag="wtm")
                    nc.sync.dma_start(
                        wo_sb[:].rearrange("p (a m) -> p a m", a=ND),
                        Wo[l][:].rearrange("(a p) m -> p a m", p=128))
                    for d in range(ND):
                        ps = ppa.tile([128, TOK], f32, tag="acc")
                        for kd in range(ND):
                            nc.tensor.matmul(
                                ps[:],
                                lhsT=wo_sb[:, kd * D + d * 128:kd * D + (d + 1) * 128],
                                rhs=onf[kd][:], start=(kd == 0),
                                stop=(kd == ND - 1))
                        nc.vector.tensor_tensor(h[d][:], h[d][:], ps[:],
                                                op=ALU.add)

                    # ---- ln2 -> x2 (reuse x tiles); FFN ----
                    ln_fm(h, ln2g[l], ln2b[l], x)
                    FH = FF // 2
                    NFH = NF // 2
                    psf = [ppa.tile([128, TOK], f32, tag="acc", name=f"psf{l}_{i}")
                           for i in range(ND)]
                    for hf in range(2):
                        w1h = wff.tile([128, ND * FH], f32r, tag="w1h")
                        nc.sync.dma_start(
                            w1h[:].rearrange("p (a m) -> p a m", a=ND),
                            W1[l][:, hf * FH:(hf + 1) * FH].rearrange(
                                "(a p) m -> p a m", p=128))
                        w2h = wff.tile([128, ND * FH], f32r, tag="w2h")
                        nc.sync.dma_start(
                            w2h[:].rearrange("p (a m) -> p a m", a=ND),
                            W2[l][:, hf * FH:(hf + 1) * FH].rearrange(
                                "(a p) m -> p a m", p=128))
                        for fi in range(NFH):
                            ft = hf * NFH + fi
                            ps1 = ppm.tile([128, TOK], f32, tag="mm")
                            for kd in range(ND):
                                nc.tensor.matmul(
                                    ps1[:],
                                    lhsT=w1h[:, kd * FH + fi * 128:kd * FH + (fi + 1) * 128],
                                    rhs=x[kd][:], start=(kd == 0),
                                    stop=(kd == ND - 1))
                            ps2 = ppm.tile([128, TOK], f32, tag="mm")
                            for kd in range(ND):
                                nc.tensor.matmul(
                                    ps2[:],
                                    lhsT=w2h[:, kd * FH + fi * 128:kd * FH + (fi + 1) * 128],
                                    rhs=x[kd][:], start=(kd == 0),
                                    stop=(kd == ND - 1))
                            sl = tp.tile([128, TOK], f32, tag="ta")
                            nc.scalar.activation(sl[:], ps1[:], AF.Silu)
                            gt = tp.tile([128, TOK], f32r, tag="tr1")
                            nc.vector.tensor_tensor(gt[:], sl[:], ps2[:],
                                                    op=ALU.mult)
                            wf_t = wfo.tile([128, D], f32r, tag="wfo")
                            nc.sync.dma_start(
                                wf_t[:], Wf[l][ft * 128:(ft + 1) * 128, :])
                            for d in range(ND):
                                nc.tensor.matmul(
                                    psf[d][:],
                                    lhsT=wf_t[:, d * 128:(d + 1) * 128],
                                    rhs=gt[:], start=(ft == 0),
                                    stop=(ft == NF - 1))
                    for d in range(ND):
                        nc.vector.tensor_tensor(h[d][:], h[d][:], psf[d][:],
                                                op=ALU.add)

            # ---------- final LN + concepts + AllGather + logits ----------
            hd1cm = tc.tile_pool(name="head1", bufs=1)
            hd1 = hd1cm.__enter__()
            hd2cm = tc.tile_pool(name="head2", bufs=2)
            hd2 = hd2cm.__enter__()
            xo = [bg.tile([128, TOK], f32r, tag=f"x{d}", name=f"xo{d}") for d in range(ND)]
            ln_fm(h, lnog, lnob, xo)
            encw_sb = cp.tile([128, ND * CD], f32r)
            nc.sync.dma_start(
                encw_sb[:].rearrange("p (a m) -> p a m", a=ND),
                encW[:].rearrange("(a p) m -> p a m", p=128))
            encb_sb = cp.tile([CD, 1], f32)
            nc.sync.dma_start(encb_sb[:], encB[:])
            ps_cc = ppm.tile([CD, TOK], f32, tag="mm")
            for kd in range(ND):
                nc.tensor.matmul(ps_cc[:],
                                 lhsT=encw_sb[:, kd * CD:(kd + 1) * CD],
                                 rhs=xo[kd][:], start=(kd == 0),
                                 stop=(kd == ND - 1))
            conc_l = tp.tile([CD, TOK], f32, tag="ta")
            nc.scalar.activation(conc_l[:], ps_cc[:], AF.Identity, bias=encb_sb[:])
            cc_in = dr.tile([CD, TOK], f32, tag="ccin")
            nc.gpsimd.dma_start(cc_in[:], conc_l[:])
            cc_out = dr.tile([NC * CD, TOK], f32, tag="ccout")
            if "cc" not in ABL:
                nc.gpsimd.collective_compute(
                    "AllGather", ALU.bypass, replica_groups=GROUPS8,
                    ins=[cc_in.opt()], outs=[cc_out.opt()])
            conc_x = hd1.tile([CD + 1, B * T], f32r, tag="concx")
            for c in range(NC):
                stg = tp.tile([CD, TOK], f32, tag="ta")
                nc.sync.dma_start(stg[:], cc_out[c * CD:(c + 1) * CD, :])
                nc.vector.tensor_copy(conc_x[:CD, c * TOK:(c + 1) * TOK],
                                      stg[:])
            nc.sync.dma_start(conc_x[CD:CD + 1, :], onesrow4k[:])
            dec_sb2 = hd1.tile([CD + 1, VS], f32r, tag="decsb")
            nc.sync.dma_start(dec_sb2[:CD, :], decW[:])
            nc.sync.dma_start(dec_sb2[CD:CD + 1, :], decB[:])
            for tt in range(0 if "head" in ABL else NTT):
                lst = hd2.tile([128, VS], f32, tag="lst")
                for vc in range(NVC):
                    ps = ppa.tile([128, VCW], f32, tag="acc")
                    nc.tensor.matmul(
                        ps[:], lhsT=conc_x[:, tt * 128:(tt + 1) * 128],
                        rhs=dec_sb2[:, vc * VCW:(vc + 1) * VCW],
                        start=True, stop=True)
                    if vc % 2 == 0:
                        nc.vector.tensor_copy(lst[:, vc * VCW:(vc + 1) * VCW],
                                              ps[:])
                    else:
                        nc.scalar.activation(lst[:, vc * VCW:(vc + 1) * VCW],
                                             ps[:], AF.Copy)
                nc.sync.dma_start(logits[tt * 128:(tt + 1) * 128, :], lst[:])
            hd2cm.__exit__(None, None, None)
            hd1cm.__exit__(None, None, None)

    nc.finalize()
    return nc


_NC_CACHE = None


def _get_nc():
    global _NC_CACHE
    if _NC_CACHE is None:
        _NC_CACHE = _build()
    return _NC_CACHE


def kernel(**inputs):
    f = np.float32
    idx = np.asarray(inputs["idx"])
    embed = np.asarray(inputs["embed"], f)
    tri = np.triu(np.ones((128, 128), f))
    base = {
        "embed": embed,
        "tri": tri,
        "onescol": np.ones((128, 1), f),
        "onerow": np.ones((1, 128), f),
        "onessq": np.ones((128, 128), f),
        "onesrow4k": np.ones((1, B * T), f),
        "lnig": np.broadcast_to(np.asarray(inputs["ln_in_g"], f)[None, :],
                                (128, D)).copy(),
        "lnib": np.broadcast_to(np.asarray(inputs["ln_in_b"], f)[None, :],
                                (128, D)).copy(),
        "lnog": np.asarray(inputs["ln_out_g"], f).reshape(D, 1),
        "lnob": np.asarray(inputs["ln_out_b"], f).reshape(D, 1),
        "encW": np.asarray(inputs["enc_W"], f),
        "encB": np.asarray(inputs["enc_b"], f).reshape(CD, 1),
    }
    for l in range(L):
        base[f"Wr{l}"] = np.asarray(inputs["Wr"][l], f)
        base[f"Wk{l}"] = np.asarray(inputs["Wk"][l], f)
        base[f"Wv{l}"] = np.asarray(inputs["Wv"][l], f)
        base[f"Wo{l}"] = np.asarray(inputs["Wo_tm"][l], f)
        base[f"W1{l}"] = np.asarray(inputs["W1"][l], f)
        base[f"W2{l}"] = np.asarray(inputs["W2"][l], f)
        base[f"Wf{l}"] = np.asarray(inputs["Wo_ff"][l], f)
        base[f"ln1g{l}"] = np.asarray(inputs["ln1_g"][l], f).reshape(D, 1)
        base[f"ln1b{l}"] = np.asarray(inputs["ln1_b"][l], f).reshape(D, 1)
        base[f"ln2g{l}"] = np.asarray(inputs["ln2_g"][l], f).reshape(D, 1)
        base[f"ln2b{l}"] = np.asarray(inputs["ln2_b"][l], f).reshape(D, 1)
        base[f"gng{l}"] = np.broadcast_to(
            np.asarray(inputs["gn_g"][l], f)[None, :], (128, D)).copy()
        base[f"gnb{l}"] = np.broadcast_to(
            np.asarray(inputs["gn_b"][l], f)[None, :], (128, D)).copy()
        base[f"dec{l}"] = np.broadcast_to(
            np.asarray(inputs["decay"][l], f)[None, :], (128, D)).copy()
    dec_W = np.asarray(inputs["dec_W"], f)
    dec_b = np.asarray(inputs["dec_b"], f)

    in_maps = []
    for c in range(NC):
        b = c // GP
        r = c % GP
        s = r * TOK
        m = dict(base)
        m["idxs"] = idx[b, s:s + TOK].astype(np.int32).reshape(TOK, 1)
        m["tglob"] = (s + np.arange(TOK, dtype=f)).reshape(NT, 128).T.copy()
        m["gmask"] = (np.arange(GP) < r).astype(f).reshape(GP, 1)
        m["decW"] = dec_W[:, c * VS:(c + 1) * VS].copy()
        m["decB"] = dec_b[c * VS:(c + 1) * VS].reshape(1, VS).copy()
        in_maps.append(m)

    nc = _get_nc()
    res = run_bass_kernel_spmd(nc, in_maps, core_ids=list(range(NC)))
    if res.exec_time_ns is not None:
        print(f"HW exec time: {res.exec_time_ns} ns", flush=True)
    out = np.concatenate([res.results[c]["logits"] for c in range(NC)], axis=1)
    return out.reshape(B, T, V).astype(np.float32)


# revision 33
# speedup vs baseline: 1.0583x; 1.0034x over previous
"""ConceptRWKV Trainium2 kernel (8 NeuronCores, SPMD).

Sharding:
  - Trunk: sequence-parallel. Core c handles batch c//4, token chunk
    (c%4)*512. Residual h kept feature-major [D(4x128 part-tiles), 512 tok]
    in SBUF float32r. Per-layer cross-core exchange: decay-cumsum carry
    (subgroup AllGather of local totals + mask matmul) and GroupNorm
    global stats (subgroup AllReduce).
  - Head: V-sharded. concepts [64, 4096] AllGathered across all 8 cores;
    each core computes logits for its own 4000-column V slice.
Host reassembles logits by concatenating V slices.
"""

import numpy as np

import concourse.bass as bass
import concourse.mybir as mybir
import concourse.tile as tile
from concourse import bacc
from concourse.bass_utils import run_bass_kernel_spmd
from concourse.masks import make_identity

B, T, V, D, FF, L, CD = 2, 2048, 32000, 512, 2048, 6, 64
EPS = 1e-5
NC = 8
GP = 4          # cores per batch group
TOK = 512       # tokens per core
NT = TOK // 128  # token tiles per core
ND = D // 128    # d tiles
NF = FF // 128   # ff tiles
VS = V // NC     # vocab slice per core
NVC = 8
VCW = VS // NVC  # 500, one psum bank
NTT = (B * T) // 128  # 32 token tiles over the full stream

f32 = mybir.dt.float32
f32r = mybir.dt.float32r
i32 = mybir.dt.int32
AF = mybir.ActivationFunctionType
ALU = mybir.AluOpType

GROUPS4 = [[0, 1, 2, 3], [4, 5, 6, 7]]
GROUPS8 = [list(range(NC))]


def _build():
    import os
    ABL = set(os.environ.get("RWKV_ABLATE", "").split(","))
    nc = bacc.Bacc("TRN2", target_bir_lowering=False, debug=False,
                   num_devices=NC)

    # ---- parameters ----
    embed = nc.declare_dram_parameter("embed", [V, D], f32, isOutput=False)
    idxs = nc.declare_dram_parameter("idxs", [TOK, 1], i32, isOutput=False)
    tglob = nc.declare_dram_parameter("tglob", [128, NT], f32, isOutput=False)
    gmask = nc.declare_dram_parameter("gmask", [GP, 1], f32r, isOutput=False)
    tri = nc.declare_dram_parameter("tri", [128, 128], f32r, isOutput=False)
    onescol = nc.declare_dram_parameter("onescol", [128, 1], f32r, isOutput=False)
    onerow = nc.declare_dram_parameter("onerow", [1, 128], f32r, isOutput=False)
    onessq = nc.declare_dram_parameter("onessq", [128, 128], f32r, isOutput=False)
    onesrow4k = nc.declare_dram_parameter("onesrow4k", [1, B * T], f32r,
                                          isOutput=False)
    lnig = nc.declare_dram_parameter("lnig", [128, D], f32, isOutput=False)
    lnib = nc.declare_dram_parameter("lnib", [128, D], f32, isOutput=False)
    lnog = nc.declare_dram_parameter("lnog", [D, 1], f32, isOutput=False)
    lnob = nc.declare_dram_parameter("lnob", [D, 1], f32, isOutput=False)
    encW = nc.declare_dram_parameter("encW", [D, CD], f32r, isOutput=False)
    encB = nc.declare_dram_parameter("encB", [CD, 1], f32, isOutput=False)
    decW = nc.declare_dram_parameter("decW", [CD, VS], f32r, isOutput=False)
    decB = nc.declare_dram_parameter("decB", [1, VS], f32r, isOutput=False)

    Wr, Wk, Wv, Wo, W1, W2, Wf = [], [], [], [], [], [], []
    ln1g, ln1b, ln2g, ln2b, gng, gnb, decs = [], [], [], [], [], [], []
    for l in range(L):
        Wr.append(nc.declare_dram_parameter(f"Wr{l}", [D, D], f32r, isOutput=False))
        Wk.append(nc.declare_dram_parameter(f"Wk{l}", [D, D], f32r, isOutput=False))
        Wv.append(nc.declare_dram_parameter(f"Wv{l}", [D, D], f32r, isOutput=False))
        Wo.append(nc.declare_dram_parameter(f"Wo{l}", [D, D], f32r, isOutput=False))
        W1.append(nc.declare_dram_parameter(f"W1{l}", [D, FF], f32r, isOutput=False))
        W2.append(nc.declare_dram_parameter(f"W2{l}", [D, FF], f32r, isOutput=False))
        Wf.append(nc.declare_dram_parameter(f"Wf{l}", [FF, D], f32r, isOutput=False))
        ln1g.append(nc.declare_dram_parameter(f"ln1g{l}", [D, 1], f32, isOutput=False))
        ln1b.append(nc.declare_dram_parameter(f"ln1b{l}", [D, 1], f32, isOutput=False))
        ln2g.append(nc.declare_dram_parameter(f"ln2g{l}", [D, 1], f32, isOutput=False))
        ln2b.append(nc.declare_dram_parameter(f"ln2b{l}", [D, 1], f32, isOutput=False))
        gng.append(nc.declare_dram_parameter(f"gng{l}", [128, D], f32, isOutput=False))
        gnb.append(nc.declare_dram_parameter(f"gnb{l}", [128, D], f32, isOutput=False))
        decs.append(nc.declare_dram_parameter(f"dec{l}", [128, D], f32, isOutput=False))

    logits = nc.declare_dram_parameter("logits", [B * T, VS], f32, isOutput=True)

    with tile.TileContext(nc) as tc:
        with (
            tc.tile_pool(name="const", bufs=1) as cp,
            tc.tile_pool(name="persist", bufs=1) as hp,
            tc.tile_pool(name="big", bufs=1) as bg,
            tc.tile_pool(name="tmp", bufs=3) as tp,
            tc.tile_pool(name="rows", bufs=1) as rp,
            tc.tile_pool(name="ppmm", bufs=2, space="PSUM") as ppm,
            tc.tile_pool(name="ppacc", bufs=4, space="PSUM") as ppa,
            tc.tile_pool(name="ppst", bufs=2, space="PSUM") as pps,
            tc.tile_pool(name="dram", bufs=1, space="DRAM") as dr,
        ):
            # ---- constants ----
            ident = cp.tile([128, 128], f32)
            make_identity(nc, ident[:])
            tri_sb = cp.tile([128, 128], f32r)
            nc.sync.dma_start(tri_sb[:], tri[:])
            ones_sq = cp.tile([128, 128], f32r)
            nc.sync.dma_start(ones_sq[:], onessq[:])
            ones_col = cp.tile([128, 1], f32r)
            nc.sync.dma_start(ones_col[:], onescol[:])
            one_row = cp.tile([1, 128], f32r)
            nc.sync.dma_start(one_row[:], onerow[:])
            gmask_sb = cp.tile([GP, 1], f32r)
            nc.sync.dma_start(gmask_sb[:], gmask[:])
            tglob_sb = cp.tile([128, NT], f32)
            nc.sync.dma_start(tglob_sb[:], tglob[:])
            epsc = cp.tile([128, 1], f32)
            nc.vector.memset(epsc[:], EPS)

            # persistent residual stream, feature-major f32r
            h = [hp.tile([128, TOK], f32r, tag=f"h{d}", name=f"h{d}") for d in range(ND)]

            # ---------- helpers ----------
            def ln_fm(hin, g_dram, b_dram, out_tiles):
                """LayerNorm over D (partitions) of feature-major hin -> f32r."""
                # per-token sum / sumsq via ones-matmuls
                ps_s = pps.tile([1, TOK], f32, tag="st")
                ps_q = pps.tile([1, TOK], f32, tag="st")
                for d in range(ND):
                    nc.tensor.matmul(ps_s[:], lhsT=ones_col[:], rhs=hin[d][:],
                                     start=(d == 0), stop=(d == ND - 1))
                for d in range(ND):
                    hsq = tp.tile([128, TOK], f32r, tag="tr1")
                    nc.gpsimd.tensor_tensor(hsq[:], hin[d][:], hin[d][:],
                                             op=ALU.mult)
                    nc.tensor.matmul(ps_q[:], lhsT=ones_col[:], rhs=hsq[:],
                                     start=(d == 0), stop=(d == ND - 1))
                mean_r = rp.tile([1, TOK], f32, tag="mean")
                nc.vector.tensor_scalar_mul(mean_r[:], ps_s[:], 1.0 / D)
                msq_r = rp.tile([1, TOK], f32, tag="msq")
                nc.vector.tensor_scalar_mul(msq_r[:], ps_q[:], 1.0 / D)
                m2_r = rp.tile([1, TOK], f32, tag="m2")
                nc.vector.tensor_tensor(m2_r[:], mean_r[:], mean_r[:], op=ALU.mult)
                var_r = rp.tile([1, TOK], f32, tag="var")
                nc.vector.tensor_tensor(var_r[:], msq_r[:], m2_r[:],
                                        op=ALU.subtract)
                nc.scalar.activation(var_r[:], var_r[:], AF.Sqrt, bias=epsc[:1, :])
                rstd_r = rp.tile([1, TOK], f32, tag="rstd")
                nc.vector.reciprocal(rstd_r[:], var_r[:])
                # A = rstd, Bc = -mean*rstd   (f32r rows for K=1 bcast matmul)
                a_r = rp.tile([1, TOK], f32r, tag="arow")
                nc.vector.tensor_copy(a_r[:], rstd_r[:])
                b_t = rp.tile([1, TOK], f32, tag="btmp")
                nc.vector.tensor_tensor(b_t[:], mean_r[:], rstd_r[:], op=ALU.mult)
                b_r = rp.tile([1, TOK], f32r, tag="brow")
                nc.vector.tensor_scalar_mul(b_r[:], b_t[:], -1.0)
                ps_a = pps.tile([128, TOK], f32, tag="st")
                nc.tensor.matmul(ps_a[:], lhsT=one_row[:], rhs=a_r[:],
                                 start=True, stop=True)
                ps_b = pps.tile([128, TOK], f32, tag="st")
                nc.tensor.matmul(ps_b[:], lhsT=one_row[:], rhs=b_r[:],
                                 start=True, stop=True)
                for d in range(ND):
                    gc = rp.tile([128, 1], f32, tag="gcol")
                    nc.sync.dma_start(gc[:], g_dram[d * 128:(d + 1) * 128, :])
                    bc = rp.tile([128, 1], f32, tag="bcol")
                    nc.sync.dma_start(bc[:], b_dram[d * 128:(d + 1) * 128, :])
                    t1 = tp.tile([128, TOK], f32, tag="ta")
                    nc.vector.tensor_tensor(t1[:], hin[d][:], ps_a[:], op=ALU.mult)
                    t2 = tp.tile([128, TOK], f32, tag="tb")
                    nc.vector.tensor_tensor(t2[:], t1[:], ps_b[:], op=ALU.add)
                    nc.scalar.activation(out_tiles[d][:], t2[:], AF.Identity,
                                         bias=bc[:], scale=gc[:])

            # ---------- embedding + input LN ----------
            x = [bg.tile([128, TOK], f32r, tag=f"x{d}", name=f"x{d}") for d in range(ND)]
            lnig_sb = bg.tile([128, D], f32, tag="gfull")
            nc.sync.dma_start(lnig_sb[:], lnig[:])
            lnib_sb = bg.tile([128, D], f32, tag="bfull")
            nc.sync.dma_start(lnib_sb[:], lnib[:])
            for tt in range(NT):
                idx_t = tp.tile([128, 1], i32, tag="idxt")
                nc.sync.dma_start(idx_t[:], idxs[tt * 128:(tt + 1) * 128, :])
                emb = tp.tile([128, D], f32, tag="ta")
                nc.gpsimd.indirect_dma_start(
                    out=emb[:], out_offset=None, in_=embed[:],
                    in_offset=bass.IndirectOffsetOnAxis(ap=idx_t[:, :1], axis=0))
                s1 = rp.tile([128, 1], f32, tag="es1")
                nc.vector.tensor_reduce(s1[:], emb[:], axis=mybir.AxisListType.X,
                                        op=ALU.add)
                nc.vector.tensor_scalar_mul(s1[:], s1[:], 1.0 / D)
                sq = tp.tile([128, D], f32, tag="tb")
                nc.vector.tensor_tensor(sq[:], emb[:], emb[:], op=ALU.mult)
                s2 = rp.tile([128, 1], f32, tag="es2")
                nc.vector.tensor_reduce(s2[:], sq[:], axis=mybir.AxisListType.X,
                                        op=ALU.add)
                nc.vector.tensor_scalar_mul(s2[:], s2[:], 1.0 / D)
                m2 = rp.tile([128, 1], f32, tag="em2")
                nc.vector.tensor_tensor(m2[:], s1[:], s1[:], op=ALU.mult)
                vv = rp.tile([128, 1], f32, tag="evv")
                nc.vector.tensor_tensor(vv[:], s2[:], m2[:], op=ALU.subtract)
                nc.scalar.activation(vv[:], vv[:], AF.Sqrt, bias=epsc[:])
                rs = rp.tile([128, 1], f32, tag="ers")
                nc.vector.reciprocal(rs[:], vv[:])
                xn = tp.tile([128, D], f32, tag="tc")
                nc.vector.tensor_scalar(xn[:], emb[:], scalar1=s1[:],
                                        scalar2=rs[:], op0=ALU.subtract,
                                        op1=ALU.mult)
                xg = tp.tile([128, D], f32, tag="tb")
                nc.vector.tensor_tensor(xg[:], xn[:], lnig_sb[:], op=ALU.mult)
                h0 = tp.tile([128, D], f32, tag="ta")
                nc.vector.tensor_tensor(h0[:], xg[:], lnib_sb[:], op=ALU.add)
                for d in range(ND):
                    ptr = ppm.tile([128, 128], f32, tag="mm")
                    nc.tensor.transpose(ptr[:], in_=h0[:, d * 128:(d + 1) * 128],
                                        identity=ident[:])
                    nc.vector.tensor_copy(h[d][:, tt * 128:(tt + 1) * 128], ptr[:])

            # ---------- layers ----------
            with (
                tc.tile_pool(name="wtm", bufs=2) as wtm,
                tc.tile_pool(name="wff", bufs=2) as wff,
                tc.tile_pool(name="wfo", bufs=3) as wfo,
            ):
                for l in range(0 if "layers" in ABL else L):
                    # ---- ln1 -> x ----
                    ln_fm(h, ln1g[l], ln1b[l], x)

                    # ---- r, k, v token-major; kv product ----
                    wr_sb = wtm.tile([128, ND * D], f32r, tag="wtm")
                    nc.sync.dma_start(
                        wr_sb[:].rearrange("p (a m) -> p a m", a=ND),
                        Wr[l][:].rearrange("(a p) m -> p a m", p=128))
                    r_sb = bg.tile([128, NT * D], f32, tag="r")
                    for tt in range(NT):
                        ps = ppm.tile([128, D], f32, tag="mm")
                        for kd in range(ND):
                            nc.tensor.matmul(
                                ps[:], lhsT=x[kd][:, tt * 128:(tt + 1) * 128],
                                rhs=wr_sb[:, kd * D:(kd + 1) * D],
                                start=(kd == 0), stop=(kd == ND - 1))
                        nc.scalar.activation(r_sb[:, tt * D:(tt + 1) * D], ps[:],
                                             AF.Sigmoid)
                    wk_sb = wtm.tile([128, ND * D], f32r, tag="wtm")
                    nc.sync.dma_start(
                        wk_sb[:].rearrange("p (a m) -> p a m", a=ND),
                        Wk[l][:].rearrange("(a p) m -> p a m", p=128))
                    k_sb = bg.tile([128, NT * D], f32r, tag="ksb")
                    for tt in range(NT):
                        ps = ppm.tile([128, D], f32, tag="mm")
                        for kd in range(ND):
                            nc.tensor.matmul(
                                ps[:], lhsT=x[kd][:, tt * 128:(tt + 1) * 128],
                                rhs=wk_sb[:, kd * D:(kd + 1) * D],
                                start=(kd == 0), stop=(kd == ND - 1))
                        nc.scalar.activation(k_sb[:, tt * D:(tt + 1) * D], ps[:],
                                             AF.Identity)
                    wv_sb = wtm.tile([128, ND * D], f32r, tag="wtm")
                    nc.sync.dma_start(
                        wv_sb[:].rearrange("p (a m) -> p a m", a=ND),
                        Wv[l][:].rearrange("(a p) m -> p a m", p=128))
                    kv_sb = k_sb
                    for tt in range(NT):
                        ps = ppm.tile([128, D], f32, tag="mm")
                        for kd in range(ND):
                            nc.tensor.matmul(
                                ps[:], lhsT=x[kd][:, tt * 128:(tt + 1) * 128],
                                rhs=wv_sb[:, kd * D:(kd + 1) * D],
                                start=(kd == 0), stop=(kd == ND - 1))
                        nc.vector.tensor_tensor(kv_sb[:, tt * D:(tt + 1) * D],
                                                k_sb[:, tt * D:(tt + 1) * D],
                                                ps[:], op=ALU.mult)

                    # ---- decay scale factors (token-major, fp32 exact) ----
                    dec_sb = tp.tile([128, D], f32, tag="ta")
                    nc.sync.dma_start(dec_sb[:], decs[l][:])
                    nc.scalar.activation(dec_sb[:], dec_sb[:], AF.Sigmoid)
                    nc.vector.tensor_scalar_max(dec_sb[:], dec_sb[:], 1e-7)
                    ld_sb = tp.tile([128, D], f32, tag="ld")
                    nc.scalar.activation(ld_sb[:], dec_sb[:], AF.Ln)
                    scale_sb = bg.tile([128, NT * D], f32, tag="scale")
                    term_sb = k_sb
                    for tt in range(NT):
                        blk = slice(tt * D, (tt + 1) * D)
                        nc.vector.tensor_scalar_mul(scale_sb[:, blk], ld_sb[:],
                                                    tglob_sb[:, tt:tt + 1])
                        nc.scalar.activation(scale_sb[:, blk], scale_sb[:, blk],
                                             AF.Exp)
                        inv_t = tp.tile([128, D], f32, tag="tc")
                        nc.vector.tensor_scalar_max(inv_t[:], scale_sb[:, blk],
                                                    1e-10)
                        nc.vector.reciprocal(inv_t[:], inv_t[:])
                        nc.vector.tensor_tensor(term_sb[:, blk],
                                                kv_sb[:, blk], inv_t[:],
                                                op=ALU.mult)

                    # ---- cumsum over tokens (tri matmul) + cross-core carry ----
                    cum = [ppa.tile([128, D], f32, tag="acc", name=f"cum{l}_{i}")
                           for i in range(NT)]
                    for tt in range(NT):
                        for ss in range(tt):
                            nc.tensor.matmul(cum[tt][:], lhsT=ones_sq[:],
                                             rhs=term_sb[:, ss * D:(ss + 1) * D],
                                             start=(ss == 0), stop=False)
                        nc.tensor.matmul(cum[tt][:], lhsT=tri_sb[:],
                                         rhs=term_sb[:, tt * D:(tt + 1) * D],
                                         start=(tt == 0), stop=False)
                    ps_tot = pps.tile([1, D], f32, tag="st")
                    for ss in range(NT):
                        nc.tensor.matmul(ps_tot[:], lhsT=ones_col[:],
                                         rhs=term_sb[:, ss * D:(ss + 1) * D],
                                         start=(ss == 0), stop=(ss == NT - 1))
                    tot_r = rp.tile([1, D], f32, tag="totr")
                    nc.scalar.activation(tot_r[:], ps_tot[:], AF.Copy)
                    cin = dr.tile([1, D], f32, tag="cin")
                    nc.gpsimd.dma_start(cin[:], tot_r[:])
                    cout = dr.tile([GP, D], f32, tag="cout")
                    if "cc" not in ABL:
                        nc.gpsimd.collective_compute(
                            "AllGather", ALU.bypass, replica_groups=GROUPS4,
                            ins=[cin.opt()], outs=[cout.opt()])
                    gath = tp.tile([GP, D], f32, tag="gath")
                    nc.gpsimd.dma_start(gath[:], cout[:])
                    gath_r = tp.tile([GP, D], f32r, tag="gathr")
                    nc.vector.tensor_copy(gath_r[:], gath[:])
                    ps_c = pps.tile([1, D], f32, tag="st")
                    nc.tensor.matmul(ps_c[:], lhsT=gmask_sb[:], rhs=gath_r[:],
                                     start=True, stop=True)
                    carry_r = rp.tile([1, D], f32r, tag="carry")
                    nc.vector.tensor_copy(carry_r[:], ps_c[:])
                    for tt in range(NT):
                        nc.tensor.matmul(cum[tt][:], lhsT=one_row[:],
                                         rhs=carry_r[:], start=False, stop=True)

                    # ---- out = r * cum * scale;  GroupNorm stats ----
                    out_sb = scale_sb
                    rsums = []
                    qsums = []
                    for tt in range(NT):
                        blk = slice(tt * D, (tt + 1) * D)
                        o1 = tp.tile([128, D], f32, tag="ta")
                        nc.vector.tensor_tensor(o1[:], scale_sb[:, blk],
                                                cum[tt][:], op=ALU.mult)
                        nc.vector.tensor_tensor(out_sb[:, blk], o1[:],
                                                r_sb[:, blk], op=ALU.mult)
                        rs_ = rp.tile([128, 1], f32, tag=f"grs{tt}")
                        nc.vector.tensor_reduce(rs_[:], out_sb[:, blk],
                                                axis=mybir.AxisListType.X,
                                                op=ALU.add)
                        rsums.append(rs_)
                        sq_ = tp.tile([128, D], f32, tag="tb")
                        nc.gpsimd.tensor_tensor(sq_[:], out_sb[:, blk],
                                                out_sb[:, blk], op=ALU.mult)
                        qs_ = rp.tile([128, 1], f32, tag=f"gqs{tt}")
                        nc.vector.tensor_reduce(qs_[:], sq_[:],
                                                axis=mybir.AxisListType.X,
                                                op=ALU.add)
                        qsums.append(qs_)
                    for lst in (rsums, qsums):
                        nc.vector.tensor_tensor(lst[0][:], lst[0][:], lst[1][:],
                                                op=ALU.add)
                        nc.vector.tensor_tensor(lst[2][:], lst[2][:], lst[3][:],
                                                op=ALU.add)
                        nc.vector.tensor_tensor(lst[0][:], lst[0][:], lst[2][:],
                                                op=ALU.add)
                    rtot = rp.tile([128, 1], f32r, tag="rtot")
                    nc.vector.tensor_copy(rtot[:], rsums[0][:])
                    qtot = rp.tile([128, 1], f32r, tag="qtot")
                    nc.vector.tensor_copy(qtot[:], qsums[0][:])
                    ps_g = pps.tile([1, 2], f32, tag="st")
                    nc.tensor.matmul(ps_g[:], lhsT=rtot[:], rhs=ones_sq[:, 0:2],
                                     start=True, stop=True)
                    ps_g2 = pps.tile([1, 2], f32, tag="st")
                    nc.tensor.matmul(ps_g2[:], lhsT=qtot[:], rhs=ones_sq[:, 0:2],
                                     start=True, stop=True)
                    st_sb = rp.tile([1, 2], f32, tag="stsb")
                    nc.scalar.activation(st_sb[:, 0:1], ps_g[:, 0:1], AF.Copy)
                    nc.scalar.activation(st_sb[:, 1:2], ps_g2[:, 0:1], AF.Copy)
                    gn_in = dr.tile([1, 2], f32, tag="gnin")
                    nc.gpsimd.dma_start(gn_in[:], st_sb[:])
                    gn_out = dr.tile([GP, 2], f32, tag="gnout")
                    if "cc" not in ABL:
                        nc.gpsimd.collective_compute(
                            "AllGather", ALU.bypass, replica_groups=GROUPS4,
                            ins=[gn_in.opt()], outs=[gn_out.opt()])
                    gst = tp.tile([GP, 2], f32, tag="gst")
                    nc.gpsimd.dma_start(gst[:], gn_out[:])
                    gst_r = tp.tile([GP, 2], f32r, tag="gstr")
                    nc.vector.tensor_copy(gst_r[:], gst[:])
                    ps_gt = pps.tile([1, 2], f32, tag="st")
                    nc.tensor.matmul(ps_gt[:], lhsT=ones_col[:GP, :],
                                     rhs=gst_r[:], start=True, stop=True)
                    gs = rp.tile([1, 2], f32, tag="gs")
                    nc.scalar.activation(gs[:], ps_gt[:], AF.Copy)
                    NTOT = float(GP * TOK * D)
                    m_t = rp.tile([1, 1], f32, tag="mt")
                    nc.vector.tensor_scalar_mul(m_t[:], gs[:, 0:1], 1.0 / NTOT)
                    q_t = rp.tile([1, 1], f32, tag="qt")
                    nc.vector.tensor_scalar_mul(q_t[:], gs[:, 1:2], 1.0 / NTOT)
                    mm_ = rp.tile([1, 1], f32, tag="mm_")
                    nc.vector.tensor_tensor(mm_[:], m_t[:], m_t[:], op=ALU.mult)
                    nc.vector.tensor_tensor(q_t[:], q_t[:], mm_[:],
                                            op=ALU.subtract)
                    nc.scalar.activation(q_t[:], q_t[:], AF.Sqrt, bias=epsc[:1, :])
                    nc.vector.reciprocal(q_t[:], q_t[:])
                    mr2 = rp.tile([1, 2], f32r, tag="mr2")
                    nc.vector.tensor_copy(mr2[:, 0:1], m_t[:])
                    nc.vector.tensor_copy(mr2[:, 1:2], q_t[:])
                    ps_mv = pps.tile([128, 2], f32, tag="st")
                    nc.tensor.matmul(ps_mv[:], lhsT=one_row[:], rhs=mr2[:],
                                     start=True, stop=True)
                    m_col = rp.tile([128, 1], f32, tag="mcol")
                    nc.vector.tensor_copy(m_col[:], ps_mv[:, 0:1])
                    v_col = rp.tile([128, 1], f32, tag="vcol")
                    nc.vector.tensor_copy(v_col[:], ps_mv[:, 1:2])
                    gng_sb = bg.tile([128, D], f32, tag="gfull")
                    nc.sync.dma_start(gng_sb[:], gng[l][:])
                    gnb_sb = bg.tile([128, D], f32, tag="bfull")
                    nc.sync.dma_start(gnb_sb[:], gnb[l][:])
                    onrm_sb = bg.tile([128, NT * D], f32, tag="r")
                    for tt in range(NT):
                        blk = slice(tt * D, (tt + 1) * D)
                        g1 = tp.tile([128, D], f32, tag="ta")
                        nc.vector.tensor_scalar(g1[:], out_sb[:, blk],
                                                scalar1=m_col[:],
                                                scalar2=v_col[:],
                                                op0=ALU.subtract, op1=ALU.mult)
                        g2 = tp.tile([128, D], f32, tag="tb")
                        nc.vector.tensor_tensor(g2[:], g1[:], gng_sb[:],
                                                op=ALU.mult)
                        nc.vector.tensor_tensor(onrm_sb[:, blk], g2[:],
                                                gnb_sb[:], op=ALU.add)

                    # ---- transpose out_norm to feature-major; h += onrm @ Wo ----
                    onf = [bg.tile([128, TOK], f32r, tag=f"onf{d}", name=f"onf{l}_{d}")
                           for d in range(ND)]
                    for tt in range(NT):
                        for d in range(ND):
                            ptr = ppm.tile([128, 128], f32, tag="mm")
                            nc.tensor.transpose(
                                ptr[:],
                                in_=onrm_sb[:, tt * D + d * 128:tt * D + (d + 1) * 128],
                                identity=ident[:])
                            nc.vector.tensor_copy(
                                onf[d][:, tt * 128:(tt + 1) * 128], ptr[:])
                    wo_sb = wtm.tile([128, ND * D], f32r, tag="wtm")
                    nc.sync.dma_start(
                        wo_sb[:].rearrange("p (a m) -> p a m", a=ND),
                        Wo[l][:].rearrange("(a p) m -> p a m", p=128))
                    for d in range(ND):
                        ps = ppa.tile([128, TOK], f32, tag="acc")
                        for kd in range(ND):
                            nc.tensor.matmul(
                                ps[:],
                                lhsT=wo_sb[:, kd * D + d * 128:kd * D + (d + 1) * 128],
                                rhs=onf[kd][:], start=(kd == 0),
                                stop=(kd == ND - 1))
                        nc.vector.tensor_tensor(h[d][:], h[d][:], ps[:],
                                                op=ALU.add)

                    # ---- ln2 -> x2 (reuse x tiles); FFN ----
                    ln_fm(h, ln2g[l], ln2b[l], x)
                    FH = FF // 2
                    NFH = NF // 2
                    psf = [ppa.tile([128, TOK], f32, tag="acc", name=f"psf{l}_{i}")
                           for i in range(ND)]
                    for hf in range(2):
                        w1h = wff.tile([128, ND * FH], f32r, tag="w1h")
                        nc.sync.dma_start(
                            w1h[:].rearrange("p (a m) -> p a m", a=ND),
                            W1[l][:, hf * FH:(hf + 1) * FH].rearrange(
                                "(a p) m -> p a m", p=128))
                        w2h = wff.tile([128, ND * FH], f32r, tag="w2h")
                        nc.sync.dma_start(
                            w2h[:].rearrange("p (a m) -> p a m", a=ND),
                            W2[l][:, hf * FH:(hf + 1) * FH].rearrange(
                                "(a p) m -> p a m", p=128))
                        for fi in range(NFH):
                            ft = hf * NFH + fi
                            ps1 = ppm.tile([128, TOK], f32, tag="mm")
                            for kd in range(ND):
                                nc.tensor.matmul(
                                    ps1[:],
                                    lhsT=w1h[:, kd * FH + fi * 128:kd * FH + (fi + 1) * 128],
                                    rhs=x[kd][:], start=(kd == 0),
                                    stop=(kd == ND - 1))
                            ps2 = ppm.tile([128, TOK], f32, tag="mm")
                            for kd in range(ND):
                                nc.tensor.matmul(
                                    ps2[:],
                                    lhsT=w2h[:, kd * FH + fi * 128:kd * FH + (fi + 1) * 128],
                                    rhs=x[kd][:], start=(kd == 0),
                                    stop=(kd == ND - 1))
                            sl = tp.tile([128, TOK], f32, tag="ta")
                            nc.scalar.activation(sl[:], ps1[:], AF.Silu)
                            gt = tp.tile([128, TOK], f32r, tag="tr1")
                            nc.vector.tensor_tensor(gt[:], sl[:], ps2[:],
                                                    op=ALU.mult)
                            wf_t = wfo.tile([128, D], f32r, tag="wfo")
                            nc.sync.dma_start(
                                wf_t[:], Wf[l][ft * 128:(ft + 1) * 128, :])
                            for d in range(ND):
                                nc.tensor.matmul(
                                    psf[d][:],
                                    lhsT=wf_t[:, d * 128:(d + 1) * 128],
                                    rhs=gt[:], start=(ft == 0),
                                    stop=(ft == NF - 1))
                    for d in range(ND):
                        nc.vector.tensor_tensor(h[d][:], h[d][:], psf[d][:],
                                                op=ALU.add)

            # ---------- final LN + concepts + AllGather + logits ----------
            hd1cm = tc.tile_pool(name="head1", bufs=1)
            hd1 = hd1cm.__enter__()
            hd2cm = tc.tile_pool(name="head2", bufs=2)
            hd2 = hd2cm.__enter__()
            xo = [bg.tile([128, TOK], f32r, tag=f"x{d}", name=f"xo{d}") for d in range(ND)]
            ln_fm(h, lnog, lnob, xo)
            encw_sb = cp.tile([128, ND * CD], f32r)
            nc.sync.dma_start(
                encw_sb[:].rearrange("p (a m) -> p a m", a=ND),
                encW[:].rearrange("(a p) m -> p a m", p=128))
            encb_sb = cp.tile([CD, 1], f32)
            nc.sync.dma_start(encb_sb[:], encB[:])
            ps_cc = ppm.tile([CD, TOK], f32, tag="mm")
            for kd in range(ND):
                nc.tensor.matmul(ps_cc[:],
                                 lhsT=encw_sb[:, kd * CD:(kd + 1) * CD],
                                 rhs=xo[kd][:], start=(kd == 0),
                                 stop=(kd == ND - 1))
            conc_l = tp.tile([CD, TOK], f32, tag="ta")
            nc.scalar.activation(conc_l[:], ps_cc[:], AF.Identity, bias=encb_sb[:])
            cc_in = dr.tile([CD, TOK], f32, tag="ccin")
            nc.gpsimd.dma_start(cc_in[:], conc_l[:])
            cc_out = dr.tile([NC * CD, TOK], f32, tag="ccout")
            if "cc" not in ABL:
                nc.gpsimd.collective_compute(
                    "AllGather", ALU.bypass, replica_groups=GROUPS8,
                    ins=[cc_in.opt()], outs=[cc_out.opt()])
            conc_x = hd1.tile([CD + 1, B * T], f32r, tag="concx")
            for c in range(NC):
                stg = tp.tile([CD, TOK], f32, tag="ta")
                nc.sync.dma_start(stg[:], cc_out[c * CD:(c + 1) * CD, :])
                nc.vector.tensor_copy(conc_x[:CD, c * TOK:(c + 1) * TOK],
                                      stg[:])
            nc.sync.dma_start(conc_x[CD:CD + 1, :], onesrow4k[:])
            dec_sb2 = hd1.tile([CD + 1, VS], f32r, tag="decsb")
            nc.sync.dma_start(dec_sb2[:CD, :], decW[:])
            nc.sync.dma_start(dec_sb2[CD:CD + 1, :], decB[:])
            for tt in range(0 if "head" in ABL else NTT):
                lst = hd2.tile([128, VS], f32, tag="lst")
                for vc in range(NVC):
                    ps = ppa.tile([128, VCW], f32, tag="acc")
                    nc.tensor.matmul(
                        ps[:], lhsT=conc_x[:, tt * 128:(tt + 1) * 128],
                        rhs=dec_sb2[:, vc * VCW:(vc + 1) * VCW],
                        start=True, stop=True)
                    if vc % 2 == 0:
                        nc.vector.tensor_copy(lst[:, vc * VCW:(vc + 1) * VCW],
                                              ps[:])
                    else:
                        nc.scalar.activation(lst[:, vc * VCW:(vc + 1) * VCW],
                                             ps[:], AF.Copy)
                nc.sync.dma_start(logits[tt * 128:(tt + 1) * 128, :], lst[:])
            hd2cm.__exit__(None, None, None)
            hd1cm.__exit__(None, None, None)

    nc.finalize()
    return nc


_NC_CACHE = None


def _get_nc():
    global _NC_CACHE
    if _NC_CACHE is None:
        _NC_CACHE = _build()
    return _NC_CACHE


def kernel(**inputs):
    f = np.float32
    idx = np.asarray(inputs["idx"])
    embed = np.asarray(inputs["embed"], f)
    tri = np.triu(np.ones((128, 128), f))
    base = {
        "embed": embed,
        "tri": tri,
        "onescol": np.ones((128, 1), f),
        "onerow": np.ones((1, 128), f),
        "onessq": np.ones((128, 128), f),
        "onesrow4k": np.ones((1, B * T), f),
        "lnig": np.broadcast_to(np.asarray(inputs["ln_in_g"], f)[None, :],
                                (128, D)).copy(),
        "lnib": np.broadcast_to(np.asarray(inputs["ln_in_b"], f)[None, :],
                                (128, D)).copy(),
        "lnog": np.asarray(inputs["ln_out_g"], f).reshape(D, 1),
        "lnob": np.asarray(inputs["ln_out_b"], f).reshape(D, 1),
        "encW": np.asarray(inputs["enc_W"], f),
        "encB": np.asarray(inputs["enc_b"], f).reshape(CD, 1),
    }
    for l in range(L):
        base[f"Wr{l}"] = np.asarray(inputs["Wr"][l], f)
        base[f"Wk{l}"] = np.asarray(inputs["Wk"][l], f)
        base[f"Wv{l}"] = np.asarray(inputs["Wv"][l], f)
        base[f"Wo{l}"] = np.asarray(inputs["Wo_tm"][l], f)
        base[f"W1{l}"] = np.asarray(inputs["W1"][l], f)
        base[f"W2{l}"] = np.asarray(inputs["W2"][l], f)
        base[f"Wf{l}"] = np.asarray(inputs["Wo_ff"][l], f)
        base[f"ln1g{l}"] = np.asarray(inputs["ln1_g"][l], f).reshape(D, 1)
        base[f"ln1b{l}"] = np.asarray(inputs["ln1_b"][l], f).reshape(D, 1)
        base[f"ln2g{l}"] = np.asarray(inputs["ln2_g"][l], f).reshape(D, 1)
        base[f"ln2b{l}"] = np.asarray(inputs["ln2_b"][l], f).reshape(D, 1)
        base[f"gng{l}"] = np.broadcast_to(
            np.asarray(inputs["gn_g"][l], f)[None, :], (128, D)).copy()
        base[f"gnb{l}"] = np.broadcast_to(
            np.asarray(inputs["gn_b"][l], f)[None, :], (128, D)).copy()
        base[f"dec{l}"] = np.broadcast_to(
            np.asarray(inputs["decay"][l], f)[None, :], (128, D)).copy()
    dec_W = np.asarray(inputs["dec_W"], f)
    dec_b = np.asarray(inputs["dec_b"], f)

    in_maps = []
    for c in range(NC):
        b = c // GP
        r = c % GP
        s = r * TOK
        m = dict(base)
        m["idxs"] = idx[b, s:s + TOK].astype(np.int32).reshape(TOK, 1)
        m["tglob"] = (s + np.arange(TOK, dtype=f)).reshape(NT, 128).T.copy()
        m["gmask"] = (np.arange(GP) < r).astype(f).reshape(GP, 1)
        m["decW"] = dec_W[:, c * VS:(c + 1) * VS].copy()
        m["decB"] = dec_b[c * VS:(c + 1) * VS].reshape(1, VS).copy()
        in_maps.append(m)

    nc = _get_nc()
    res = run_bass_kernel_spmd(nc, in_maps, core_ids=list(range(NC)))
    if res.exec_time_ns is not None:
        print(f"HW exec time: {res.exec_time_ns} ns", flush=True)
    out = np.concatenate([res.results[c]["logits"] for c in range(NC)], axis=1)
    return out.reshape(B, T, V).astype(np.float32)
